# revision 17
# baseline (speedup 1.0000x reference)
"""Trainium2 Bass kernel for DiT focused-linear-attention block (nn_DiT_9259949490457).

Data-parallel over batch: 16 batches -> 8 NeuronCores, 2 batches/core, no collectives.

v2: q and kv GEMMs run in fp8-e4m3 hi/lo split-precision using DoubleRow perf mode
(2 slice-products per 0.5-cycle/row instruction -> 14 DR units vs 18 bf16 units per
K=1152 contraction, with better-than-bf16 accuracy). The hi/lo fp8 cast of x is fused
into the transpose-psum evacuation (ACT writes hi, DVE subtract writes lo). The
depthwise 3x3 conv branch pairs (dy=-1,dy=+1) taps into DoubleRow fp8 matmuls via
custom overlapping access patterns. Per-batch phase order: KV -> einsum1 -> Q ->
norms -> dwconv -> einsum2+proj so the dwconv matmuls fill the norm-chain PE gap.

Self-contained: hardcodes all shapes; host numpy pre-packs fp8 hi/lo weights
(scaled by 64; 1/64 folded into psum-evacuation activations).
"""

import numpy as np
import ml_dtypes
import bass_rust

import concourse.bacc as bacc
import concourse.mybir as mybir
import concourse.tile as tile
from concourse import bass_utils

F32 = mybir.dt.float32
BF16 = mybir.dt.bfloat16
FP8 = mybir.dt.float8e4
ALU = mybir.AluOpType
AF = mybir.ActivationFunctionType
AX = mybir.AxisListType
DR = mybir.MatmulPerfMode.DoubleRow

NCORES = 8
B, N, DIM = 16, 1024, 1152
H, KVH, HD = 12, 4, 96
BL = B // NCORES          # 2 local batches
T = BL * N                # 2048 local tokens
NK = DIM // 128           # 9 feature K-slices
TT = N // 128             # 8 token tiles per batch
CH = N // 512             # 2 free-dim chunks of 512 per batch
SW = 64.0                 # fp8 weight pre-scale (power of two)

_BF = ml_dtypes.bfloat16
_F8 = ml_dtypes.float8_e4m3fn


def _spanp(b):
    if b % 128 == 0:
        return 128
    if b % 64 == 0:
        return 64
    return 32


def _head_pieces(h):
    out = []
    rr = 0
    while rr < 96:
        gr = 96 * h + rr
        j, r0 = divmod(gr, 128)
        cnt = min(96 - rr, 128 - r0, _spanp(r0), _spanp(rr))
        out.append((j, r0, rr, cnt))
        rr += cnt
    return out


def _ins_dim(ap, stride, count):
    """Insert a free dim [stride, count] right after the partition dim."""
    dims = [list(d) for d in ap.ap]
    new = [dims[0], [stride, count]] + dims[1:]
    return bass_rust.AP(ap.tensor, ap.offset, new)


def _build_kernel():
    nc = bacc.Bacc("TRN2", target_bir_lowering=False, debug=False,
                   enable_asserts=True, num_devices=NCORES)
    x_in = nc.dram_tensor("x", [T, DIM], F32, kind="ExternalInput").ap()
    wqfj_in = nc.dram_tensor("wqfj", [128, NK, 2, NK, 128], FP8, kind="ExternalInput").ap()
    wq8r_in = nc.dram_tensor("wq8r", [128, 2, DIM], FP8, kind="ExternalInput").ap()
    wkvf_in = nc.dram_tensor("wkvf", [128, 2, NK, 768], FP8, kind="ExternalInput").ap()
    wkv8r_in = nc.dram_tensor("wkv8r", [128, 2, 768], FP8, kind="ExternalInput").ap()
    pwT_in = nc.dram_tensor("pwT", [DIM, DIM], BF16, kind="ExternalInput").ap()
    wqb_in = nc.dram_tensor("wqb", [128, NK], F32, kind="ExternalInput").ap()
    kvb64_in = nc.dram_tensor("kvb64", [1, 768], BF16, kind="ExternalInput").ap()
    pjb_bc_in = nc.dram_tensor("pjb_bc", [1, DIM], BF16, kind="ExternalInput").ap()
    dgp_in = nc.dram_tensor("dgp", [96, 2, KVH, 3, 96], FP8, kind="ExternalInput").ap()
    dge_in = nc.dram_tensor("dge", [96, KVH, 9, 96], FP8, kind="ExternalInput").ap()
    dwcb_in = nc.dram_tensor("dwcb", [96, KVH], F32, kind="ExternalInput").ap()
    masks_in = nc.dram_tensor("masks", [128, NK, H], BF16, kind="ExternalInput").ap()
    eye_in = nc.dram_tensor("eye", [128, 128], BF16, kind="ExternalInput").ap()
    y_out = nc.dram_tensor("y", [T, DIM], F32, kind="ExternalOutput").ap()

    from contextlib import ExitStack
    with tile.TileContext(nc) as tc, ExitStack() as stack:
        cpool = stack.enter_context(tc.tile_pool(name="const", bufs=1))
        dpool = stack.enter_context(tc.tile_pool(name="dram", bufs=1, space="DRAM"))

        # ---- consts / weights ----
        eye = cpool.tile([128, 128], BF16, name="eye")
        nc.sync.dma_start(out=eye[:], in_=eye_in[:])
        wkvf = cpool.tile([128, 2, NK, 768], FP8, name="wkvf")
        wkv8r = cpool.tile([128, 2, 768], FP8, name="wkv8r")
        kvb64 = cpool.tile([1, 768], BF16, name="kvb64")
        wqfj = cpool.tile([128, NK, 2, NK, 128], FP8, name="wqfj")
        wq8r = cpool.tile([128, 2, DIM], FP8, name="wq8r")
        wqb = cpool.tile([128, NK], F32, name="wqb")
        PWT = [cpool.tile([128, DIM], BF16, name=f"PWT{k}") for k in range(NK)]
        pjb_bc = cpool.tile([1, DIM], BF16, name="pjb_bc")
        dgp = cpool.tile([96, 2, KVH, 3, 96], FP8, name="dgp")
        dge = cpool.tile([96, KVH, 9, 96], FP8, name="dge")
        dwcb = cpool.tile([96, KVH], F32, name="dwcb")
        masks = cpool.tile([128, NK, H], BF16, name="masks")
        ones_r = cpool.tile([1, 128], BF16, name="ones_r")
        nc.vector.memset(ones_r[:], 1.0)
        ones_c = cpool.tile([128, 1], BF16, name="ones_c")
        nc.vector.memset(ones_c[:], 1.0)

        vpad = dpool.tile([BL, N, KVH, 128], BF16, name="vpad")

        # ---- pools ----
        xpool = stack.enter_context(tc.tile_pool(name="xf", bufs=1))
        XF = xpool.tile([128, 2, NK, T], FP8, name="XF")
        wp = stack.enter_context(tc.tile_pool(name="work", bufs=1))
        pmm = stack.enter_context(tc.tile_pool(name="pmm", bufs=1, space="PSUM"))
        pa = stack.enter_context(tc.tile_pool(name="pa", bufs=1, space="PSUM"))

        # ---- prologue: load x, transpose on PE, evacuate as fp8 hi/lo into XF ----
        with tc.tile_pool(name="prep", bufs=3) as prep:
            for i in range(T // 128):
                stage = prep.tile([128, DIM], BF16, name="stage", tag="stage")
                nc.gpsimd.dma_start(out=stage[:],
                                    in_=x_in[128 * i:128 * (i + 1), :])
                pt_a = pa.tile([128, 1024], BF16, name="pt_a", tag="pa", bufs=2)
                for k in range(8):
                    nc.tensor.transpose(pt_a[:, 128 * k:128 * (k + 1)],
                                        stage[:, 128 * k:128 * (k + 1)], eye[:])
                pt_b = pa.tile([128, 128], BF16, name="pt_b", tag="pa", bufs=2)
                nc.tensor.transpose(pt_b[:], stage[:, 1024:1152], eye[:])
                cs = slice(128 * i, 128 * (i + 1))
                pt3 = pt_a[:].rearrange("p (k c) -> p k c", c=128)
                # hi casts (ACT), slices 0-7 slot1, slice 8 slot0
                nc.scalar.activation(XF[:, 1, 0:8, cs], pt3, AF.Copy)
                nc.scalar.activation(XF[:, 0, 8, cs], pt_b[:], AF.Copy)
                # lo = psum - hi (DVE), slices 0-7 slot0, slice 8 slot1
                nc.vector.tensor_tensor(out=XF[:, 0, 0:8, cs], in0=pt3,
                                        in1=XF[:, 1, 0:8, cs], op=ALU.subtract)
                nc.vector.tensor_tensor(out=XF[:, 1, 8, cs], in0=pt_b[:],
                                        in1=XF[:, 0, 8, cs], op=ALU.subtract)
                if i == 2:
                    # stagger weight loads behind the first x tiles
                    nc.sync.dma_start(out=wkvf[:], in_=wkvf_in[:])
                    nc.sync.dma_start(out=wkv8r[:], in_=wkv8r_in[:])
                    nc.sync.dma_start(out=kvb64[:], in_=kvb64_in[:])
                elif i == 5:
                    for j in range(NK):
                        nc.sync.dma_start(out=wqfj[:, j], in_=wqfj_in[:, j])
                    nc.sync.dma_start(out=wq8r[:], in_=wq8r_in[:])
                    nc.sync.dma_start(out=wqb[:], in_=wqb_in[:])
                elif i == 9:
                    for k in range(NK):
                        nc.sync.dma_start(out=PWT[k][:],
                                          in_=pwT_in[128 * k:128 * (k + 1), :])
                    nc.sync.dma_start(out=pjb_bc[:], in_=pjb_bc_in[:])
                elif i == 11:
                    nc.sync.dma_start(out=dgp[:], in_=dgp_in[:])
                    nc.sync.dma_start(out=dge[:], in_=dge_in[:])
                    nc.sync.dma_start(out=dwcb[:], in_=dwcb_in[:])
                    nc.sync.dma_start(out=masks[:], in_=masks_in[:])

        def wkv_ap(kind, a, cols):
            cc, w = cols
            if kind == "pair":
                return wkvf[:, 0, 2 * a:2 * a + 2, cc:cc + w]
            return wkvf[:, :, a, cc:cc + w]

        def wkv8r_ap(cols):
            cc, w = cols
            return wkv8r[:, :, cc:cc + w]

        for b in range(BL):
            _emit_batch(nc, tc, b, wp, pmm, pa, wqfj, wq8r, wkv_ap, wkv8r_ap, PWT,
                        wqb, kvb64, pjb_bc, dgp, dge, dwcb, masks, ones_r, ones_c,
                        XF, vpad, y_out)

    nc.compile()
    return nc


def _emit_batch(nc, tc, b, wp, pmm, pa, wqfj, wq8r, wkv_ap, wkv8r_ap, PWT, wqb,
                kvb64, pjb_bc, dgp, dge, dwcb, masks, ones_r, ones_c, XF, vpad,
                y_out):
    # ---------------- phase K: fp8 DR GEMM + focus(k) ----------------
    acc2k = wp.tile([128, KVH * TT], F32, name="acc2k", tag="acc2k")
    k3 = [wp.tile([128, 384], BF16, name=f"k3_{t}", tag=f"k3_{t}") for t in range(TT)]
    vv = [wp.tile([128, 384], BF16, name=f"v_{t}", tag=f"v_{t}") for t in range(TT)]

    # stationary = XF token slices, moving = wkv weight columns
    def emit_kv_half(t, lo, out_pk):
        t0 = b * N + 128 * t
        for ci, cc in ((0, lo), (192, lo + 192)):
            dst = out_pk[:, ci:ci + 192]
            for a in range(4):
                nc.tensor.matmul(dst, XF[:, 1, 2 * a:2 * a + 2, t0:t0 + 128],
                                 wkv_ap("pair", a, (cc, 192)),
                                 start=(a == 0), stop=False, perf_mode=DR)
            nc.tensor.matmul(dst, XF[:, :, 8, t0:t0 + 128],
                             wkv_ap("cross", 8, (cc, 192)),
                             start=False, stop=False, perf_mode=DR)
            for k in range(8):
                nc.tensor.matmul(dst, XF[:, :, k, t0:t0 + 128],
                                 wkv_ap("cross", k, (cc, 192)),
                                 start=False, stop=False, perf_mode=DR)
            nc.tensor.matmul(dst, XF[:, :, 8, t0:t0 + 128],
                             wkv8r_ap((cc, 192)),
                             start=False, stop=False, perf_mode=DR)
            nc.tensor.matmul(dst, ones_r[:], kvb64[:, cc:cc + 192],
                             start=False, stop=True)

    acc1kr = pa_norm_row = None
    for t in range(TT):
        pk = pmm.tile([128, 512], F32, name="pk", tag="pmm", bufs=3)
        emit_kv_half(t, 0, pk)
        uk = wp.tile([128, 384], BF16, name="uk", tag="uk", bufs=2)
        nc.scalar.activation(uk[:], pk[:, 0:384], AF.Relu, scale=1.0 / SW)
        uk2 = wp.tile([128, 384], BF16, name="uk2", tag="uk2", bufs=2)
        nc.scalar.activation(uk2[:], uk[:], AF.Square)
        if t == 0:
            acc1kr = pmm.tile([1, 384], F32, name="acc1kr", tag="pnorm", bufs=1)
        nc.tensor.matmul(acc1kr[:], ones_c[:], uk2[:],
                         start=(t == 0), stop=(t == TT - 1))
        nc.vector.tensor_mul(k3[t][:], uk2[:], uk[:])
        uk6 = wp.tile([128, 384], BF16, name="uk6", tag="uk6", bufs=2)
        nc.vector.tensor_mul(uk6[:], k3[t][:], k3[t][:])
        for g in range(KVH):
            nc.vector.tensor_reduce(acc2k[:, g * TT + t:g * TT + t + 1],
                                    uk6[:, 96 * g:96 * (g + 1)],
                                    axis=AX.X, op=ALU.add)

    # ---------------- phase V ----------------
    for t in range(TT):
        pv = pmm.tile([128, 512], F32, name="pv", tag="pmm", bufs=3)
        emit_kv_half(t, 384, pv)
        nc.vector.tensor_scalar_mul(vv[t][:], pv[:, 0:384], 1.0 / SW)
        nc.sync.dma_start(
            out=vpad[b, 128 * t:128 * (t + 1), :, 0:96],
            in_=vv[t][:].rearrange("p (k d) -> p k d", k=KVH))

    # dwconv input chain (DMA + casts) emitted now; conv matmuls after norms
    vT8 = []
    for g in range(KVH):
        vT = wp.tile([128, N], BF16, name="vTd", tag="vTd", bufs=2)
        nc.sync.dma_start(out=vT[:], in_=vpad[b, :, g, :], transpose=True)
        v8 = wp.tile([96, N], FP8, name="v8", tag=f"v8_{g}")
        if g % 2 == 0:
            nc.scalar.activation(v8[:], vT[0:96, :], AF.Copy)
        else:
            nc.vector.tensor_copy(v8[:], vT[0:96, :])
        vT8.append(v8)

    # ---------------- einsum1 (unscaled) ----------------
    kvu = [wp.tile([96, 96], BF16, name=f"kvu_{g}", tag=f"kvu_{g}")
           for g in range(KVH)]
    for g in range(KVH):
        pk_t = pa.tile([96, 96], F32, name="pkvt", tag="pa", bufs=2)
        for t in range(TT):
            nc.tensor.matmul(pk_t[:], k3[t][:, 96 * g:96 * (g + 1)],
                             vv[t][:, 96 * g:96 * (g + 1)],
                             start=(t == 0), stop=(t == TT - 1))
        nc.vector.tensor_copy(kvu[g][:], pk_t[:])

    # ---------------- phase Q: fp8 DR GEMM + focus(q) ----------------
    acc1q = wp.tile([128, NK * CH], F32, name="acc1q", tag="acc1q")
    acc2q = wp.tile([128, NK * CH], F32, name="acc2q", tag="acc2q")
    q3 = [wp.tile([128, N], BF16, name=f"q3_{j}", tag=f"q3_{j}") for j in range(NK)]
    for c2 in range(CH):
        for j in range(NK):
            pq = pmm.tile([128, 512], F32, name="pq", tag="pmm", bufs=3)
            for sub in range(2):
                t0 = b * N + 512 * c2 + 256 * sub
                dst = pq[:, 256 * sub:256 * (sub + 1)]
                for a in range(4):
                    nc.tensor.matmul(dst, wqfj[:, j, 0, 2 * a:2 * a + 2, :],
                                     XF[:, 1, 2 * a:2 * a + 2, t0:t0 + 256],
                                     start=(a == 0), stop=False, perf_mode=DR)
                nc.tensor.matmul(dst, wqfj[:, j, :, 8, :],
                                 XF[:, :, 8, t0:t0 + 256],
                                 start=False, stop=False, perf_mode=DR)
                for k in range(8):
                    nc.tensor.matmul(dst, wqfj[:, j, :, k, :],
                                     XF[:, :, k, t0:t0 + 256],
                                     start=False, stop=False, perf_mode=DR)
                nc.tensor.matmul(dst, wq8r[:, :, 128 * j:128 * (j + 1)],
                                 XF[:, :, 8, t0:t0 + 256],
                                 start=False, stop=(sub == 1), perf_mode=DR)
            u = wp.tile([128, 512], BF16, name="u", tag="u", bufs=2)
            nc.scalar.activation(u[:], pq[:], AF.Relu, scale=1.0 / SW,
                                 bias=wqb[:, j:j + 1])
            u2 = wp.tile([128, 512], BF16, name="u2", tag="u2", bufs=2)
            col = j * CH + c2
            nc.scalar.activation(u2[:], u[:], AF.Square,
                                 accum_out=acc1q[:, col:col + 1])
            q3s = q3[j][:, 512 * c2:512 * (c2 + 1)]
            nc.vector.tensor_mul(q3s, u2[:], u[:])
            u6 = wp.tile([128, 512], BF16, name="u6", tag="u6", bufs=2)
            nc.scalar.activation(u6[:], q3s, AF.Square,
                                 accum_out=acc2q[:, col:col + 1])

    # ---------------- norms -> per-head scale g ----------------
    sq_rows = []
    for ai, acc in enumerate((acc1q, acc2q)):
        accs = wp.tile([128, NK], F32, name="accs", tag="accs", bufs=2)
        av = acc[:, 0:NK * CH].rearrange("p (j c) -> p j c", c=CH)
        nc.vector.tensor_add(accs[:], av[:, :, 0], av[:, :, 1])
        accsb = wp.tile([128, NK], BF16, name="accsb", tag="accsb", bufs=2)
        nc.vector.tensor_copy(accsb[:], accs[:])
        psn = pa.tile([1, H], F32, name="psn", tag="pa", bufs=2)
        for j in range(NK):
            nc.tensor.matmul(psn[:], accsb[:, j:j + 1], masks[:, j, :],
                             start=(j == 0), stop=(j == NK - 1))
        srow = wp.tile([1, H], F32, name="srow", tag="srow", bufs=4)
        nc.vector.tensor_copy(srow[:], psn[:])
        sq_rows.append(srow)
    sk_rows = []
    kred1 = wp.tile([1, KVH], F32, name="kred1", tag="kred1", bufs=2)
    nc.vector.tensor_reduce(kred1[:],
                            acc1kr[:].rearrange("a (k d) -> a k d", k=KVH),
                            axis=AX.X, op=ALU.add)
    sk_rows.append(kred1)
    acc2kb = wp.tile([128, KVH * TT], BF16, name="acc2kb", tag="acc2kb")
    nc.vector.tensor_copy(acc2kb[:], acc2k[:])
    psk = pa.tile([1, KVH * TT], F32, name="psk", tag="pa", bufs=2)
    nc.tensor.matmul(psk[:], ones_c[:], acc2kb[:], start=True, stop=True)
    krow = wp.tile([1, KVH * TT], F32, name="krow", tag="krow", bufs=2)
    nc.vector.tensor_copy(krow[:], psk[:])
    kred2 = wp.tile([1, KVH], F32, name="kred2", tag="kred2", bufs=2)
    nc.vector.tensor_reduce(kred2[:], krow[:].rearrange("a (k t) -> a k t", k=KVH),
                            axis=AX.X, op=ALU.add)
    sk_rows.append(kred2)

    def _f_row(s1, s2, width, tagp):
        se = wp.tile([1, width], F32, name="se", tag=f"se{tagp}", bufs=2)
        nc.vector.tensor_scalar_add(se[:], s2[:], 1e-30)
        rc = wp.tile([1, width], F32, name="rc", tag=f"rc{tagp}", bufs=2)
        nc.vector.reciprocal(rc[:], se[:])
        rt = wp.tile([1, width], F32, name="rt", tag=f"rt{tagp}", bufs=2)
        nc.vector.tensor_mul(rt[:], s1[:], rc[:])
        fr = wp.tile([1, width], F32, name="fr", tag=f"fr{tagp}", bufs=2)
        nc.scalar.activation(fr[:], rt[:], AF.Sqrt)
        return fr

    fq = _f_row(sq_rows[0], sq_rows[1], H, "q")
    fk = _f_row(sk_rows[0], sk_rows[1], KVH, "k")
    fk12 = wp.tile([1, H], F32, name="fk12", tag="fk12")
    for g in range(3):
        nc.vector.tensor_copy(fk12[:, 4 * g:4 * (g + 1)], fk[:])
    grow = wp.tile([1, H], F32, name="grow", tag="grow")
    nc.vector.tensor_mul(grow[:], fq[:], fk12[:])
    gb = wp.tile([96, H], F32, name="gb", tag="gb")
    nc.gpsimd.partition_broadcast(gb[:], grow[:], channels=96)

    # ---------------- dwconv matmuls (fp8, DR tap pairs) ----------------
    vdwc = [wp.tile([96, N], BF16, name=f"vdwc_{g}", tag=f"vdwc_{g}")
            for g in range(KVH)]
    for g in range(KVH):
        v3 = vT8[g][:].rearrange("p (y x) -> p y x", y=32)
        for hf in range(2):
            pd = pmm.tile([96, 512], F32, name="pd", tag="pdw", bufs=2)
            p3 = pd[:].rearrange("p (y x) -> p y x", y=16)
            mms = []
            # dy=0 singles (plain fp8)
            for dxi, dx in enumerate((-1, 0, 1)):
                x0, x1 = max(0, -dx), 32 - max(0, dx)
                mms.append((dge[:, g, 3 + dxi, :],
                            v3[0:96, 16 * hf:16 * hf + 16, x0 + dx:x1 + dx],
                            p3[:, 0:16, x0:x1], None))
            # (dy=-1, dy=+1) DR pairs
            ya0 = max(1, 16 * hf)
            ya1 = min(31, 16 * hf + 16)
            for dxi, dx in enumerate((-1, 0, 1)):
                x0, x1 = max(0, -dx), 32 - max(0, dx)
                base = v3[0:96, ya0 - 1:ya1 - 1, x0 + dx:x1 + dx]
                rhs = _ins_dim(base, 64, 2)
                mms.append((dgp[:, :, g, dxi, :], rhs,
                            p3[:, ya0 - 16 * hf:ya1 - 16 * hf, x0:x1], DR))
            # y-edge singles
            for dxi, dx in enumerate((-1, 0, 1)):
                x0, x1 = max(0, -dx), 32 - max(0, dx)
                if hf == 0:  # y=0, tap dy=+1
                    mms.append((dge[:, g, 6 + dxi, :],
                                v3[0:96, 1:2, x0 + dx:x1 + dx],
                                p3[:, 0:1, x0:x1], None))
                else:        # y=31, tap dy=-1
                    mms.append((dge[:, g, dxi, :],
                                v3[0:96, 30:31, x0 + dx:x1 + dx],
                                p3[:, 15:16, x0:x1], None))
            for mi, (lhsT, rhs, out, pm) in enumerate(mms):
                nc.tensor.matmul(out, lhsT, rhs, start=(mi == 0),
                                 stop=(mi == len(mms) - 1), perf_mode=pm)
            nc.scalar.activation(vdwc[g][:, 512 * hf:512 * (hf + 1)], pd[:],
                                 AF.Identity, scale=1.0 / SW,
                                 bias=dwcb[:, g:g + 1])

    # ---------------- scale kv by per-head g ----------------
    kvp = [wp.tile([96, 96], BF16, name=f"kvp_{h}", tag=f"kvp_{h}")
           for h in range(H)]
    for h in range(H):
        nc.vector.tensor_scalar_mul(kvp[h][:], kvu[h % KVH][:], gb[:, h:h + 1])

    # ---------------- einsum2 + combine into OT, then proj ----------------
    OT = [wp.tile([128, N], BF16, name=f"OT_{j}", tag=f"OT_{j}") for j in range(NK)]
    for c2 in range(CH):
        for h in range(H):
            pieces = _head_pieces(h)
            if len(pieces) == 1:
                j0, r00, _, _ = pieces[0]
                rhs = q3[j0][r00:r00 + 96, 512 * c2:512 * (c2 + 1)]
            else:
                qh = wp.tile([96, 512], BF16, name="qh", tag="qh", bufs=4)
                for pi, (j, r0, rr, cnt) in enumerate(pieces):
                    src_ap = q3[j][r0:r0 + cnt, 512 * c2:512 * (c2 + 1)]
                    nc.vector.tensor_copy(qh[rr:rr + cnt, :], src_ap)
                rhs = qh[:]
            pa_t = pa.tile([96, 512], F32, name="pat", tag="pa", bufs=2)
            nc.tensor.matmul(pa_t[:], kvp[h][:], rhs, start=True, stop=True)
            if len(pieces) == 1:
                # aligned head: fuse psum + vdwc -> OT in one DVE op
                j0, r00, _, _ = pieces[0]
                nc.vector.tensor_tensor(
                    out=OT[j0][r00:r00 + 96, 512 * c2:512 * (c2 + 1)],
                    in0=pa_t[:],
                    in1=vdwc[h % KVH][:, 512 * c2:512 * (c2 + 1)],
                    op=ALU.add)
            else:
                pac = wp.tile([96, 512], BF16, name="pac", tag="pac", bufs=4)
                nc.scalar.copy(pac[:], pa_t[:])
                for (j, r0, rr, cnt) in pieces:
                    nc.vector.tensor_tensor(
                        out=OT[j][r0:r0 + cnt, 512 * c2:512 * (c2 + 1)],
                        in0=pac[rr:rr + cnt, :],
                        in1=vdwc[h % KVH][rr:rr + cnt, 512 * c2:512 * (c2 + 1)],
                        op=ALU.add)
    for t in range(TT):
        for oc in range(3):
            py = pmm.tile([128, 384], F32, name="py", tag="pdw", bufs=2)
            for j in range(NK):
                nc.tensor.matmul(py[:], OT[j][:, 128 * t:128 * (t + 1)],
                                 PWT[j][:, 384 * oc:384 * (oc + 1)],
                                 start=(j == 0), stop=False)
            nc.tensor.matmul(py[:], ones_r[:], pjb_bc[:, 384 * oc:384 * (oc + 1)],
                             start=False, stop=True)
            ysb = wp.tile([128, 384], F32, name="ysb", tag="ysb", bufs=3)
            if (t + oc) % 2 == 0:
                nc.scalar.copy(ysb[:], py[:])
            else:
                nc.vector.tensor_copy(ysb[:], py[:])
            t0 = b * N + 128 * t
            nc.sync.dma_start(out=y_out[t0:t0 + 128, 384 * oc:384 * (oc + 1)],
                              in_=ysb[:])


_NC_CACHE = None


def _get_nc():
    global _NC_CACHE
    if _NC_CACHE is None:
        _NC_CACHE = _build_kernel()
    return _NC_CACHE


def _hi_lo(a):
    hi = a.astype(_F8)
    lo = (a - hi.astype(np.float32)).astype(_F8)
    return hi, lo


def _host_consts(wq_w, wq_b, wkv_w, wkv_b, dwc_w, dwc_b, proj_w, proj_b):
    wqT = np.ascontiguousarray(np.asarray(wq_w, np.float32).T) * SW      # [in, out]
    wkvT = np.ascontiguousarray(np.asarray(wkv_w, np.float32).T) * SW    # [in, 768]
    qhi, qlo = _hi_lo(wqT)
    khi, klo = _hi_lo(wkvT)

    # wqfj: [128, j, slot(hi,lo), k, 128]
    wqfj = np.zeros((128, NK, 2, NK, 128), _F8)
    for k in range(NK):
        for j in range(NK):
            wqfj[:, j, 0, k, :] = qhi[128 * k:128 * (k + 1), 128 * j:128 * (j + 1)]
            wqfj[:, j, 1, k, :] = qlo[128 * k:128 * (k + 1), 128 * j:128 * (j + 1)]
    wq8r = np.zeros((128, 2, DIM), _F8)
    wq8r[:, 0, :] = qlo[128 * 8:, :]
    wq8r[:, 1, :] = qhi[128 * 8:, :]

    wkvf = np.zeros((128, 2, NK, 768), _F8)
    for k in range(NK):
        wkvf[:, 0, k, :] = khi[128 * k:128 * (k + 1), :]
        wkvf[:, 1, k, :] = klo[128 * k:128 * (k + 1), :]
    wkv8r = np.zeros((128, 2, 768), _F8)
    wkv8r[:, 0, :] = klo[128 * 8:, :]
    wkv8r[:, 1, :] = khi[128 * 8:, :]

    pwT = np.ascontiguousarray(np.asarray(proj_w, np.float32).T).astype(_BF)
    wqb = np.ascontiguousarray(np.asarray(wq_b, np.float32).reshape(NK, 128).T)
    kvb64 = (np.asarray(wkv_b, np.float32).reshape(1, 768) * SW).astype(_BF)
    pjb_bc = np.asarray(proj_b, np.float32).reshape(1, DIM).astype(_BF)

    dw = np.asarray(dwc_w, np.float32).reshape(KVH, 96, 9) * SW  # [g, d, tap]
    # dgp: [96, slot(dy=-1,dy=+1), g, dxi, 96] diagonalized
    dgp = np.zeros((96, 2, KVH, 3, 96), np.float32)
    dge = np.zeros((96, KVH, 9, 96), np.float32)
    for d in range(96):
        for dxi in range(3):
            dgp[d, 0, :, dxi, d] = dw[:, d, 0 + dxi]       # dy=-1 row: taps 0,1,2
            dgp[d, 1, :, dxi, d] = dw[:, d, 6 + dxi]       # dy=+1 row: taps 6,7,8
        for ti in range(9):
            dge[d, :, ti, d] = dw[:, d, ti]
    dgp = dgp.astype(_F8)
    dge = dge.astype(_F8)
    dwcb = np.ascontiguousarray(np.asarray(dwc_b, np.float32).reshape(KVH, 96).T)

    mk = np.zeros((128, NK, H), np.float32)
    for j in range(NK):
        for p in range(128):
            f = 128 * j + p
            mk[p, j, f // 96] = 1.0
    masks = mk.astype(_BF)
    eye = np.eye(128, dtype=np.float32).astype(_BF)
    return dict(wqfj=wqfj, wq8r=wq8r, wkvf=wkvf, wkv8r=wkv8r, pwT=pwT, wqb=wqb,
                kvb64=kvb64, pjb_bc=pjb_bc, dgp=dgp, dge=dge, dwcb=dwcb,
                masks=masks, eye=eye)


def kernel(x, wq_w, wq_b, wkv_w, wkv_b, dwc_w, dwc_b, proj_w, proj_b,
           _want_results=False, **_unused):
    nc = _get_nc()
    consts = _host_consts(wq_w, wq_b, wkv_w, wkv_b, dwc_w, dwc_b, proj_w, proj_b)
    x = np.asarray(x, np.float32)
    in_maps = []
    for c in range(NCORES):
        m = dict(consts)
        m["x"] = np.ascontiguousarray(x[BL * c:BL * (c + 1)].reshape(T, DIM))
        in_maps.append(m)
    res = bass_utils.run_bass_kernel_spmd(nc, in_maps, core_ids=list(range(NCORES)))
    y = np.stack([res.results[c]["y"].reshape(BL, N, DIM) for c in range(NCORES)])
    y = y.reshape(B, N, DIM)
    if _want_results:
        return y, res
    return y


# revision 26
# speedup vs baseline: 1.0376x; 1.0376x over previous
"""Trainium2 Bass kernel for DiT focused-linear-attention block (nn_DiT_9259949490457).

Data-parallel over batch: 16 batches -> 8 NeuronCores, 2 batches/core, no collectives.

q and kv GEMMs run in fp8-e4m3 hi/lo split-precision using DoubleRow perf mode
(2 slice-products per 0.5-cycle/row instruction -> 14 DR units vs 18 bf16 units per
K=1152 contraction, with better-than-bf16 accuracy). The hi/lo fp8 cast of x is fused
into the transpose-psum evacuation (ACT writes hi, DVE subtract writes lo). The
depthwise 3x3 conv branch pairs (dy=-1,dy=+1) taps into DoubleRow fp8 matmuls via
custom overlapping access patterns.

The two local batches are phase-interleaved so PE always has matmul work while the
other batch's norm chain / einsum2-evacuation drains on ACT/DVE:
  b0:[K,V,vT,e1,Q,u6,dwc] b1:K b0:[norms,e2] b1:[V,vT,e1] b0:proj[0:6]
  b1:[Q,u6,dwc] b0:proj[6:8] b1:[norms,e2,proj]

Self-contained: hardcodes all shapes; host numpy pre-packs fp8 hi/lo weights
(scaled by 64; 1/64 folded into psum-evacuation activations).
"""

import numpy as np
import ml_dtypes
import bass_rust

import concourse.bacc as bacc
import concourse.mybir as mybir
import concourse.tile as tile
from concourse import bass_utils

F32 = mybir.dt.float32
BF16 = mybir.dt.bfloat16
FP8 = mybir.dt.float8e4
ALU = mybir.AluOpType
AF = mybir.ActivationFunctionType
AX = mybir.AxisListType
DR = mybir.MatmulPerfMode.DoubleRow

NCORES = 8
B, N, DIM = 16, 1024, 1152
H, KVH, HD = 12, 4, 96
BL = B // NCORES          # 2 local batches
T = BL * N                # 2048 local tokens
NK = DIM // 128           # 9 feature K-slices
TT = N // 128             # 8 token tiles per batch
CH = N // 512             # 2 free-dim chunks of 512 per batch
SW = 64.0                 # fp8 weight pre-scale (power of two)

_BF = ml_dtypes.bfloat16
_F8 = ml_dtypes.float8_e4m3fn


def _spanp(b):
    if b % 128 == 0:
        return 128
    if b % 64 == 0:
        return 64
    return 32


def _head_pieces(h):
    out = []
    rr = 0
    while rr < 96:
        gr = 96 * h + rr
        j, r0 = divmod(gr, 128)
        cnt = min(96 - rr, 128 - r0, _spanp(r0), _spanp(rr))
        out.append((j, r0, rr, cnt))
        rr += cnt
    return out


def _ins_dim(ap, stride, count):
    """Insert a free dim [stride, count] right after the partition dim."""
    dims = [list(d) for d in ap.ap]
    new = [dims[0], [stride, count]] + dims[1:]
    return bass_rust.AP(ap.tensor, ap.offset, new)


class _St:
    pass


def _build_kernel():
    nc = bacc.Bacc("TRN2", target_bir_lowering=False, debug=False,
                   enable_asserts=True, num_devices=NCORES)
    x_in = nc.dram_tensor("x", [T, DIM], F32, kind="ExternalInput").ap()
    wqfj_in = nc.dram_tensor("wqfj", [128, NK, 2, NK, 128], FP8, kind="ExternalInput").ap()
    wq8r_in = nc.dram_tensor("wq8r", [128, 2, DIM], FP8, kind="ExternalInput").ap()
    wkvfk_in = nc.dram_tensor("wkvfk", [128, 2, NK, 384], FP8, kind="ExternalInput").ap()
    wkvfv_in = nc.dram_tensor("wkvfv", [128, 2, NK, 384], FP8, kind="ExternalInput").ap()
    wkv8rk_in = nc.dram_tensor("wkv8rk", [128, 2, 384], FP8, kind="ExternalInput").ap()
    wkv8rv_in = nc.dram_tensor("wkv8rv", [128, 2, 384], FP8, kind="ExternalInput").ap()
    pwT_in = nc.dram_tensor("pwT", [DIM, DIM], BF16, kind="ExternalInput").ap()
    wqb_in = nc.dram_tensor("wqb", [128, NK], F32, kind="ExternalInput").ap()
    kvb64_in = nc.dram_tensor("kvb64", [1, 768], BF16, kind="ExternalInput").ap()
    pjb_bc_in = nc.dram_tensor("pjb_bc", [1, DIM], BF16, kind="ExternalInput").ap()
    dgp_in = nc.dram_tensor("dgp", [96, 2, KVH, 3, 96], FP8, kind="ExternalInput").ap()
    dge_in = nc.dram_tensor("dge", [96, KVH, 9, 96], FP8, kind="ExternalInput").ap()
    dwcb_in = nc.dram_tensor("dwcb", [96, KVH], F32, kind="ExternalInput").ap()
    masks_in = nc.dram_tensor("masks", [128, NK, H], BF16, kind="ExternalInput").ap()
    eye_in = nc.dram_tensor("eye", [128, 128], BF16, kind="ExternalInput").ap()
    y_out = nc.dram_tensor("y", [T, DIM], F32, kind="ExternalOutput").ap()

    from contextlib import ExitStack
    with tile.TileContext(nc) as tc, ExitStack() as stack:
        cpool = stack.enter_context(tc.tile_pool(name="const", bufs=1))
        dpool = stack.enter_context(tc.tile_pool(name="dram", bufs=1, space="DRAM"))

        # ---- consts / weights ----
        eye = cpool.tile([128, 128], BF16, name="eye")
        wkvfk = cpool.tile([128, 2, NK, 384], FP8, name="wkvfk")
        wkvfv = cpool.tile([128, 2, NK, 384], FP8, name="wkvfv")
        wkv8rk = cpool.tile([128, 2, 384], FP8, name="wkv8rk")
        wkv8rv = cpool.tile([128, 2, 384], FP8, name="wkv8rv")
        kvb64 = cpool.tile([1, 768], BF16, name="kvb64")
        wqfj = cpool.tile([128, NK, 2, NK, 128], FP8, name="wqfj")
        wq8r = cpool.tile([128, 2, DIM], FP8, name="wq8r")
        wqb = cpool.tile([128, NK], F32, name="wqb")
        PWT = [cpool.tile([128, DIM], BF16, name=f"PWT{k}") for k in range(NK)]
        pjb_bc = cpool.tile([1, DIM], BF16, name="pjb_bc")
        dgp = cpool.tile([96, 2, KVH, 3, 96], FP8, name="dgp")
        dge = cpool.tile([96, KVH, 9, 96], FP8, name="dge")
        dwcb = cpool.tile([96, KVH], F32, name="dwcb")
        masks = cpool.tile([128, NK, H], BF16, name="masks")
        ones_r = cpool.tile([1, 128], BF16, name="ones_r")
        ones_c = cpool.tile([128, 1], BF16, name="ones_c")
        sqwarm = cpool.tile([1, 8], F32, name="sqwarm")

        vpad = dpool.tile([BL, N, KVH, 128], BF16, name="vpad")

        # ---- pools ----
        xpool = stack.enter_context(tc.tile_pool(name="xf", bufs=1))
        XF = xpool.tile([128, 2, NK, T], FP8, name="XF")
        wp = stack.enter_context(tc.tile_pool(name="work", bufs=1))
        pmm = stack.enter_context(tc.tile_pool(name="pmm", bufs=1, space="PSUM"))
        pa = stack.enter_context(tc.tile_pool(name="pa", bufs=1, space="PSUM"))

        # ---- prologue: load x, transpose on PE, evacuate as fp8 hi/lo into XF ----
        with tc.tile_pool(name="prep", bufs=3) as prep:
            for i in range(T // 128):
                stage = prep.tile([128, DIM], BF16, name="stage", tag="stage")
                nc.gpsimd.dma_start(out=stage[:],
                                    in_=x_in[128 * i:128 * (i + 1), :])
                if i == 0:
                    nc.sync.dma_start(out=eye[:], in_=eye_in[:])
                    nc.vector.memset(ones_r[:], 1.0)
                    nc.vector.memset(ones_c[:], 1.0)
                    nc.vector.memset(sqwarm[:], 1.0)
                    # warm the Sqrt activation table off the critical path
                    nc.scalar.activation(sqwarm[:], sqwarm[:], AF.Sqrt)
                elif i == 1:
                    nc.sync.dma_start(out=wkvfk[:], in_=wkvfk_in[:])
                    nc.sync.dma_start(out=wkv8rk[:], in_=wkv8rk_in[:])
                    nc.sync.dma_start(out=kvb64[:], in_=kvb64_in[:])
                elif i == 3:
                    nc.sync.dma_start(out=wkvfv[:], in_=wkvfv_in[:])
                    nc.sync.dma_start(out=wkv8rv[:], in_=wkv8rv_in[:])
                elif i == 5:
                    for j in range(NK):
                        nc.sync.dma_start(out=wqfj[:, j], in_=wqfj_in[:, j])
                    nc.sync.dma_start(out=wq8r[:], in_=wq8r_in[:])
                    nc.sync.dma_start(out=wqb[:], in_=wqb_in[:])
                elif i == 9:
                    for k in range(NK):
                        nc.sync.dma_start(out=PWT[k][:],
                                          in_=pwT_in[128 * k:128 * (k + 1), :])
                    nc.sync.dma_start(out=pjb_bc[:], in_=pjb_bc_in[:])
                elif i == 11:
                    nc.sync.dma_start(out=dgp[:], in_=dgp_in[:])
                    nc.sync.dma_start(out=dge[:], in_=dge_in[:])
                    nc.sync.dma_start(out=dwcb[:], in_=dwcb_in[:])
                    nc.sync.dma_start(out=masks[:], in_=masks_in[:])
                pt_a = pa.tile([128, 1024], BF16, name="pt_a", tag="pa", bufs=2)
                for k in range(8):
                    nc.tensor.transpose(pt_a[:, 128 * k:128 * (k + 1)],
                                        stage[:, 128 * k:128 * (k + 1)], eye[:])
                pt_b = pa.tile([128, 128], BF16, name="pt_b", tag="pa", bufs=2)
                nc.tensor.transpose(pt_b[:], stage[:, 1024:1152], eye[:])
                cs = slice(128 * i, 128 * (i + 1))
                pt3 = pt_a[:].rearrange("p (k c) -> p k c", c=128)
                # hi casts (ACT), slices 0-7 slot1, slice 8 slot0
                nc.scalar.activation(XF[:, 1, 0:8, cs], pt3, AF.Copy)
                nc.scalar.activation(XF[:, 0, 8, cs], pt_b[:], AF.Copy)
                # lo = psum - hi (DVE), slices 0-7 slot0, slice 8 slot1
                nc.vector.tensor_tensor(out=XF[:, 0, 0:8, cs], in0=pt3,
                                        in1=XF[:, 1, 0:8, cs], op=ALU.subtract)
                nc.vector.tensor_tensor(out=XF[:, 1, 8, cs], in0=pt_b[:],
                                        in1=XF[:, 0, 8, cs], op=ALU.subtract)

        cn = _St()
        cn.wqfj, cn.wq8r, cn.wqb = wqfj, wq8r, wqb
        cn.wkvfk, cn.wkvfv, cn.wkv8rk, cn.wkv8rv = wkvfk, wkvfv, wkv8rk, wkv8rv
        cn.kvb64, cn.PWT, cn.pjb_bc = kvb64, PWT, pjb_bc
        cn.dgp, cn.dge, cn.dwcb, cn.masks = dgp, dge, dwcb, masks
        cn.ones_r, cn.ones_c, cn.XF, cn.vpad, cn.y_out = ones_r, ones_c, XF, vpad, y_out

        p0 = _phases(nc, 0, wp, pmm, pa, cn)
        p1 = _phases(nc, 1, wp, pmm, pa, cn)
        p0["k"](); p0["v"](); p0["vt"](); p0["e1"](); p0["q"](); p0["u6"](); p0["dwc"]()
        p1["k"]()
        p0["norms"](); p0["e2"]()
        p1["v"](); p1["vt"](); p1["e1"]()
        p0["proj"](0, 6)
        p1["q"](); p1["u6"](); p1["dwc"]()
        p0["proj"](6, 8)
        p1["norms"](); p1["e2"](); p1["proj"](0, 8)

    nc.compile()
    return nc


def _phases(nc, b, wp, pmm, pa, cn):
    st = _St()
    XF = cn.XF

    def emit_kv_half(t, vhalf, out_pk):
        t0 = b * N + 128 * t
        wf = cn.wkvfv if vhalf else cn.wkvfk
        w8 = cn.wkv8rv if vhalf else cn.wkv8rk
        for ci, cc in ((0, 0), (192, 192)):
            dst = out_pk[:, ci:ci + 192]
            for a in range(4):
                nc.tensor.matmul(dst, XF[:, 1, 2 * a:2 * a + 2, t0:t0 + 128],
                                 wf[:, 0, 2 * a:2 * a + 2, cc:cc + 192],
                                 start=(a == 0), stop=False, perf_mode=DR)
            nc.tensor.matmul(dst, XF[:, :, 8, t0:t0 + 128],
                             wf[:, :, 8, cc:cc + 192],
                             start=False, stop=False, perf_mode=DR)
            for k in range(8):
                nc.tensor.matmul(dst, XF[:, :, k, t0:t0 + 128],
                                 wf[:, :, k, cc:cc + 192],
                                 start=False, stop=False, perf_mode=DR)
            nc.tensor.matmul(dst, XF[:, :, 8, t0:t0 + 128],
                             w8[:, :, cc:cc + 192],
                             start=False, stop=False, perf_mode=DR)
            bc = 384 * vhalf + cc
            nc.tensor.matmul(dst, cn.ones_r[:], cn.kvb64[:, bc:bc + 192],
                             start=False, stop=True)

    def ph_k():
        st.k3 = [wp.tile([128, 384], BF16, name=f"k3_{t}", tag=f"k3_{t}")
                 for t in range(TT)]
        st.vv = [wp.tile([128, 384], BF16, name=f"v_{t}", tag=f"v_{t}")
                 for t in range(TT)]
        st.acc2k = wp.tile([128, KVH * TT], F32, name="acc2k", tag="acc2k", bufs=2)
        st.uk2s = []
        for t in range(TT):
            pk = pmm.tile([128, 512], F32, name="pk", tag="pmm", bufs=3)
            emit_kv_half(t, 0, pk)
            if t == 0:
                st.acc1kr = pmm.tile([1, 384], F32, name="acc1kr", tag="pnorm",
                                     bufs=1)
            if t >= 2:
                # row-accumulate acc1k at a 2-tile lag so PE never waits on ACT
                nc.tensor.matmul(st.acc1kr[:], cn.ones_c[:], st.uk2s[t - 2][:],
                                 start=(t == 2), stop=False)
            uk = wp.tile([128, 384], BF16, name="uk", tag="uk", bufs=2)
            nc.scalar.activation(uk[:], pk[:, 0:384], AF.Relu, scale=1.0 / SW)
            uk2 = wp.tile([128, 384], BF16, name="uk2", tag="uk2", bufs=3)
            st.uk2s.append(uk2)
            nc.scalar.activation(uk2[:], uk[:], AF.Square)
            nc.vector.tensor_mul(st.k3[t][:], uk2[:], uk[:])
            uk6 = wp.tile([128, 384], BF16, name="uk6", tag="uk6", bufs=2)
            nc.vector.tensor_mul(uk6[:], st.k3[t][:], st.k3[t][:])
            for g in range(KVH):
                nc.vector.tensor_reduce(st.acc2k[:, g * TT + t:g * TT + t + 1],
                                        uk6[:, 96 * g:96 * (g + 1)],
                                        axis=AX.X, op=ALU.add)

    def ph_v():
        for t in range(TT):
            pv = pmm.tile([128, 512], F32, name="pv", tag="pmm", bufs=3)
            if t < 2:
                # flush the lagged acc1k row-accumulation
                nc.tensor.matmul(st.acc1kr[:], cn.ones_c[:],
                                 st.uk2s[TT - 2 + t][:],
                                 start=False, stop=(t == 1))
            emit_kv_half(t, 1, pv)
            nc.vector.tensor_scalar_mul(st.vv[t][:], pv[:, 0:384], 1.0 / SW)
            nc.sync.dma_start(
                out=cn.vpad[b, 128 * t:128 * (t + 1), :, 0:96],
                in_=st.vv[t][:].rearrange("p (k d) -> p k d", k=KVH))
        # k-side acc1 group sums: free the pnorm psum row early
        st.kred1 = wp.tile([1, KVH], F32, name="kred1", tag="kred1", bufs=2)
        nc.vector.tensor_reduce(st.kred1[:],
                                st.acc1kr[:].rearrange("a (k d) -> a k d", k=KVH),
                                axis=AX.X, op=ALU.add)

    def ph_vt():
        st.vT8 = []
        for g in range(KVH):
            vT = wp.tile([128, N], BF16, name="vTd", tag="vTd", bufs=2)
            nc.sync.dma_start(out=vT[:], in_=cn.vpad[b, :, g, :], transpose=True)
            v8 = wp.tile([96, N], FP8, name="v8", tag=f"v8_{g}")
            if g % 2 == 0:
                nc.scalar.activation(v8[:], vT[0:96, :], AF.Copy)
            else:
                nc.vector.tensor_copy(v8[:], vT[0:96, :])
            st.vT8.append(v8)

    def ph_e1():
        st.kvu = [wp.tile([96, 96], BF16, name=f"kvu_{g}", tag=f"kvu_{g}")
                  for g in range(KVH)]
        for g in range(KVH):
            pk_t = pa.tile([96, 96], F32, name="pkvt", tag="pa", bufs=2)
            for t in range(TT):
                nc.tensor.matmul(pk_t[:], st.k3[t][:, 96 * g:96 * (g + 1)],
                                 st.vv[t][:, 96 * g:96 * (g + 1)],
                                 start=(t == 0), stop=(t == TT - 1))
            nc.vector.tensor_copy(st.kvu[g][:], pk_t[:])

    def ph_q():
        st.acc1q = wp.tile([128, NK * CH], F32, name="acc1q", tag="acc1q")
        st.acc2q = wp.tile([128, NK * CH], F32, name="acc2q", tag="acc2q")
        st.q3 = [wp.tile([128, N], BF16, name=f"q3_{j}", tag=f"q3_{j}")
                 for j in range(NK)]
        wqfj, wq8r = cn.wqfj, cn.wq8r
        for c2 in range(CH):
            for j in range(NK):
                pq = pmm.tile([128, 512], F32, name="pq", tag="pmm", bufs=3)
                for sub in range(2):
                    t0 = b * N + 512 * c2 + 256 * sub
                    dst = pq[:, 256 * sub:256 * (sub + 1)]
                    for a in range(4):
                        nc.tensor.matmul(dst, wqfj[:, j, 0, 2 * a:2 * a + 2, :],
                                         XF[:, 1, 2 * a:2 * a + 2, t0:t0 + 256],
                                         start=(a == 0), stop=False, perf_mode=DR)
                    nc.tensor.matmul(dst, wqfj[:, j, :, 8, :],
                                     XF[:, :, 8, t0:t0 + 256],
                                     start=False, stop=False, perf_mode=DR)
                    for k in range(8):
                        nc.tensor.matmul(dst, wqfj[:, j, :, k, :],
                                         XF[:, :, k, t0:t0 + 256],
                                         start=False, stop=False, perf_mode=DR)
                    nc.tensor.matmul(dst, wq8r[:, :, 128 * j:128 * (j + 1)],
                                     XF[:, :, 8, t0:t0 + 256],
                                     start=False, stop=(sub == 1), perf_mode=DR)
                u = wp.tile([128, 512], BF16, name="u", tag="u", bufs=2)
                nc.scalar.activation(u[:], pq[:], AF.Relu, scale=1.0 / SW,
                                     bias=cn.wqb[:, j:j + 1])
                u2 = wp.tile([128, 512], BF16, name="u2", tag="u2", bufs=2)
                col = j * CH + c2
                nc.scalar.activation(u2[:], u[:], AF.Square,
                                     accum_out=st.acc1q[:, col:col + 1])
                nc.vector.tensor_mul(st.q3[j][:, 512 * c2:512 * (c2 + 1)],
                                     u2[:], u[:])

    def ph_u6():
        # deferred acc2q: runs after the Q GEMMs (overlapping the dwconv
        # window) instead of blocking psum evacuation; split ACT/DVE
        for c2 in range(CH):
            for j in range(NK):
                col = j * CH + c2
                q3s = st.q3[j][:, 512 * c2:512 * (c2 + 1)]
                u6 = wp.tile([128, 512], BF16, name="u6", tag="u6", bufs=2)
                if (j + c2) % 2 == 0:
                    nc.scalar.activation(u6[:], q3s, AF.Square,
                                         accum_out=st.acc2q[:, col:col + 1])
                else:
                    nc.vector.tensor_mul(u6[:], q3s, q3s)
                    nc.vector.tensor_reduce(st.acc2q[:, col:col + 1], u6[:],
                                            axis=AX.X, op=ALU.add)

    def ph_dwc():
        st.vdwc = [wp.tile([96, N], BF16, name=f"vdwc_{g}", tag=f"vdwc_{g}")
                   for g in range(KVH)]
        for g in range(KVH):
            v3 = st.vT8[g][:].rearrange("p (y x) -> p y x", y=32)
            for hf in range(2):
                pd = pmm.tile([96, 512], F32, name="pd", tag="pdw", bufs=2)
                p3 = pd[:].rearrange("p (y x) -> p y x", y=16)
                mms = []
                for dxi, dx in enumerate((-1, 0, 1)):
                    x0, x1 = max(0, -dx), 32 - max(0, dx)
                    mms.append((cn.dge[:, g, 3 + dxi, :],
                                v3[0:96, 16 * hf:16 * hf + 16, x0 + dx:x1 + dx],
                                p3[:, 0:16, x0:x1], None))
                ya0 = max(1, 16 * hf)
                ya1 = min(31, 16 * hf + 16)
                for dxi, dx in enumerate((-1, 0, 1)):
                    x0, x1 = max(0, -dx), 32 - max(0, dx)
                    base = v3[0:96, ya0 - 1:ya1 - 1, x0 + dx:x1 + dx]
                    rhs = _ins_dim(base, 64, 2)
                    mms.append((cn.dgp[:, :, g, dxi, :], rhs,
                                p3[:, ya0 - 16 * hf:ya1 - 16 * hf, x0:x1], DR))
                for dxi, dx in enumerate((-1, 0, 1)):
                    x0, x1 = max(0, -dx), 32 - max(0, dx)
                    if hf == 0:  # y=0, tap dy=+1
                        mms.append((cn.dge[:, g, 6 + dxi, :],
                                    v3[0:96, 1:2, x0 + dx:x1 + dx],
                                    p3[:, 0:1, x0:x1], None))
                    else:        # y=31, tap dy=-1
                        mms.append((cn.dge[:, g, dxi, :],
                                    v3[0:96, 30:31, x0 + dx:x1 + dx],
                                    p3[:, 15:16, x0:x1], None))
                for mi, (lhsT, rhs, out, pm) in enumerate(mms):
                    nc.tensor.matmul(out, lhsT, rhs, start=(mi == 0),
                                     stop=(mi == len(mms) - 1), perf_mode=pm)
                nc.scalar.activation(st.vdwc[g][:, 512 * hf:512 * (hf + 1)],
                                     pd[:], AF.Identity, scale=1.0 / SW,
                                     bias=cn.dwcb[:, g:g + 1])

    def ph_norms():
        sq_rows = []
        for ai, acc in enumerate((st.acc1q, st.acc2q)):
            accs = wp.tile([128, NK], F32, name="accs", tag="accs", bufs=2)
            av = acc[:, 0:NK * CH].rearrange("p (j c) -> p j c", c=CH)
            nc.vector.tensor_add(accs[:], av[:, :, 0], av[:, :, 1])
            accsb = wp.tile([128, NK], BF16, name="accsb", tag="accsb", bufs=2)
            nc.vector.tensor_copy(accsb[:], accs[:])
            psn = pa.tile([1, H], F32, name="psn", tag="pa", bufs=2)
            for j in range(NK):
                nc.tensor.matmul(psn[:], accsb[:, j:j + 1], cn.masks[:, j, :],
                                 start=(j == 0), stop=(j == NK - 1))
            srow = wp.tile([1, H], F32, name="srow", tag="srow", bufs=4)
            nc.vector.tensor_copy(srow[:], psn[:])
            sq_rows.append(srow)
        acc2kb = wp.tile([128, KVH * TT], BF16, name="acc2kb", tag="acc2kb",
                         bufs=2)
        nc.vector.tensor_copy(acc2kb[:], st.acc2k[:])
        psk = pa.tile([1, KVH * TT], F32, name="psk", tag="pa", bufs=2)
        nc.tensor.matmul(psk[:], cn.ones_c[:], acc2kb[:], start=True, stop=True)
        krow = wp.tile([1, KVH * TT], F32, name="krow", tag="krow", bufs=2)
        nc.vector.tensor_copy(krow[:], psk[:])
        kred2 = wp.tile([1, KVH], F32, name="kred2", tag="kred2", bufs=2)
        nc.vector.tensor_reduce(kred2[:],
                                krow[:].rearrange("a (k t) -> a k t", k=KVH),
                                axis=AX.X, op=ALU.add)
        sk_rows = [st.kred1, kred2]

        def _f_row(s1, s2, width, tagp):
            se = wp.tile([1, width], F32, name="se", tag=f"se{tagp}", bufs=2)
            nc.vector.tensor_scalar_add(se[:], s2[:], 1e-30)
            rc = wp.tile([1, width], F32, name="rc", tag=f"rc{tagp}", bufs=2)
            nc.vector.reciprocal(rc[:], se[:])
            rt = wp.tile([1, width], F32, name="rt", tag=f"rt{tagp}", bufs=2)
            nc.vector.tensor_mul(rt[:], s1[:], rc[:])
            fr = wp.tile([1, width], F32, name="fr", tag=f"fr{tagp}", bufs=2)
            nc.scalar.activation(fr[:], rt[:], AF.Sqrt)
            return fr

        fq = _f_row(sq_rows[0], sq_rows[1], H, "q")
        fk = _f_row(sk_rows[0], sk_rows[1], KVH, "k")
        fk12 = wp.tile([1, H], F32, name="fk12", tag="fk12", bufs=2)
        for g in range(3):
            nc.vector.tensor_copy(fk12[:, 4 * g:4 * (g + 1)], fk[:])
        grow = wp.tile([1, H], F32, name="grow", tag="grow", bufs=2)
        nc.vector.tensor_mul(grow[:], fq[:], fk12[:])
        gb = wp.tile([96, H], F32, name="gb", tag="gb", bufs=2)
        nc.gpsimd.partition_broadcast(gb[:], grow[:], channels=96)
        st.kvp = [wp.tile([96, 96], BF16, name=f"kvp_{h}", tag=f"kvp_{h}")
                  for h in range(H)]
        for h in range(H):
            nc.vector.tensor_scalar_mul(st.kvp[h][:], st.kvu[h % KVH][:],
                                        gb[:, h:h + 1])

    def ph_e2():
        st.OT = [wp.tile([128, N], BF16, name=f"OT_{j}", tag=f"OT_{j}")
                 for j in range(NK)]
        for c2 in range(CH):
            for h in range(H):
                pieces = _head_pieces(h)
                if len(pieces) == 1:
                    j0, r00, _, _ = pieces[0]
                    rhs = st.q3[j0][r00:r00 + 96, 512 * c2:512 * (c2 + 1)]
                else:
                    qh = wp.tile([96, 512], BF16, name="qh", tag="qh", bufs=4)
                    for pi, (j, r0, rr, cnt) in enumerate(pieces):
                        src_ap = st.q3[j][r0:r0 + cnt, 512 * c2:512 * (c2 + 1)]
                        if (h + pi) % 2 == 0:
                            nc.vector.tensor_copy(qh[rr:rr + cnt, :], src_ap)
                        else:
                            nc.scalar.copy(qh[rr:rr + cnt, :], src_ap)
                    rhs = qh[:]
                pa_t = pa.tile([96, 512], F32, name="pat", tag="pa", bufs=2)
                nc.tensor.matmul(pa_t[:], st.kvp[h][:], rhs, start=True,
                                 stop=True)
                if len(pieces) == 1:
                    j0, r00, _, _ = pieces[0]
                    nc.vector.tensor_tensor(
                        out=st.OT[j0][r00:r00 + 96, 512 * c2:512 * (c2 + 1)],
                        in0=pa_t[:],
                        in1=st.vdwc[h % KVH][:, 512 * c2:512 * (c2 + 1)],
                        op=ALU.add)
                else:
                    pac = wp.tile([96, 512], BF16, name="pac", tag="pac", bufs=4)
                    nc.scalar.copy(pac[:], pa_t[:])
                    for (j, r0, rr, cnt) in pieces:
                        nc.vector.tensor_tensor(
                            out=st.OT[j][r0:r0 + cnt, 512 * c2:512 * (c2 + 1)],
                            in0=pac[rr:rr + cnt, :],
                            in1=st.vdwc[h % KVH][rr:rr + cnt,
                                                 512 * c2:512 * (c2 + 1)],
                            op=ALU.add)

    def ph_proj(ta, tb):
        for t in range(ta, tb):
            for oc in range(3):
                py = pmm.tile([128, 384], F32, name="py", tag="pdw", bufs=2)
                for j in range(NK):
                    nc.tensor.matmul(py[:], st.OT[j][:, 128 * t:128 * (t + 1)],
                                     cn.PWT[j][:, 384 * oc:384 * (oc + 1)],
                                     start=(j == 0), stop=False)
                nc.tensor.matmul(py[:], cn.ones_r[:],
                                 cn.pjb_bc[:, 384 * oc:384 * (oc + 1)],
                                 start=False, stop=True)
                ysb = wp.tile([128, 384], F32, name="ysb", tag="ysb", bufs=3)
                if (t + oc) % 2 == 0:
                    nc.scalar.copy(ysb[:], py[:])
                else:
                    nc.vector.tensor_copy(ysb[:], py[:])
                t0 = b * N + 128 * t
                nc.sync.dma_start(out=cn.y_out[t0:t0 + 128,
                                               384 * oc:384 * (oc + 1)],
                                  in_=ysb[:])

    return dict(k=ph_k, v=ph_v, vt=ph_vt, e1=ph_e1, q=ph_q, u6=ph_u6,
                dwc=ph_dwc, norms=ph_norms, e2=ph_e2, proj=ph_proj)


_NC_CACHE = None


def _get_nc():
    global _NC_CACHE
    if _NC_CACHE is None:
        _NC_CACHE = _build_kernel()
    return _NC_CACHE


def _hi_lo(a):
    hi = a.astype(_F8)
    lo = (a - hi.astype(np.float32)).astype(_F8)
    return hi, lo


def _host_consts(wq_w, wq_b, wkv_w, wkv_b, dwc_w, dwc_b, proj_w, proj_b):
    wqT = np.ascontiguousarray(np.asarray(wq_w, np.float32).T) * SW      # [in, out]
    wkvT = np.ascontiguousarray(np.asarray(wkv_w, np.float32).T) * SW    # [in, 768]
    qhi, qlo = _hi_lo(wqT)
    khi, klo = _hi_lo(wkvT)

    # wqfj: [128, j, slot(hi,lo), k, 128]
    wqfj = np.zeros((128, NK, 2, NK, 128), _F8)
    for k in range(NK):
        for j in range(NK):
            wqfj[:, j, 0, k, :] = qhi[128 * k:128 * (k + 1), 128 * j:128 * (j + 1)]
            wqfj[:, j, 1, k, :] = qlo[128 * k:128 * (k + 1), 128 * j:128 * (j + 1)]
    wq8r = np.zeros((128, 2, DIM), _F8)
    wq8r[:, 0, :] = qlo[128 * 8:, :]
    wq8r[:, 1, :] = qhi[128 * 8:, :]

    wkvf = np.zeros((128, 2, NK, 768), _F8)
    for k in range(NK):
        wkvf[:, 0, k, :] = khi[128 * k:128 * (k + 1), :]
        wkvf[:, 1, k, :] = klo[128 * k:128 * (k + 1), :]
    wkv8r = np.zeros((128, 2, 768), _F8)
    wkv8r[:, 0, :] = klo[128 * 8:, :]
    wkv8r[:, 1, :] = khi[128 * 8:, :]
    wkvfk = np.ascontiguousarray(wkvf[:, :, :, 0:384])
    wkvfv = np.ascontiguousarray(wkvf[:, :, :, 384:768])
    wkv8rk = np.ascontiguousarray(wkv8r[:, :, 0:384])
    wkv8rv = np.ascontiguousarray(wkv8r[:, :, 384:768])

    pwT = np.ascontiguousarray(np.asarray(proj_w, np.float32).T).astype(_BF)
    wqb = np.ascontiguousarray(np.asarray(wq_b, np.float32).reshape(NK, 128).T)
    kvb64 = (np.asarray(wkv_b, np.float32).reshape(1, 768) * SW).astype(_BF)
    pjb_bc = np.asarray(proj_b, np.float32).reshape(1, DIM).astype(_BF)

    dw = np.asarray(dwc_w, np.float32).reshape(KVH, 96, 9) * SW  # [g, d, tap]
    dgp = np.zeros((96, 2, KVH, 3, 96), np.float32)
    dge = np.zeros((96, KVH, 9, 96), np.float32)
    for d in range(96):
        for dxi in range(3):
            dgp[d, 0, :, dxi, d] = dw[:, d, 0 + dxi]       # dy=-1 taps 0,1,2
            dgp[d, 1, :, dxi, d] = dw[:, d, 6 + dxi]       # dy=+1 taps 6,7,8
        for ti in range(9):
            dge[d, :, ti, d] = dw[:, d, ti]
    dgp = dgp.astype(_F8)
    dge = dge.astype(_F8)
    dwcb = np.ascontiguousarray(np.asarray(dwc_b, np.float32).reshape(KVH, 96).T)

    mk = np.zeros((128, NK, H), np.float32)
    for j in range(NK):
        for p in range(128):
            f = 128 * j + p
            mk[p, j, f // 96] = 1.0
    masks = mk.astype(_BF)
    eye = np.eye(128, dtype=np.float32).astype(_BF)
    return dict(wqfj=wqfj, wq8r=wq8r, wkvfk=wkvfk, wkvfv=wkvfv, wkv8rk=wkv8rk,
                wkv8rv=wkv8rv, pwT=pwT, wqb=wqb, kvb64=kvb64, pjb_bc=pjb_bc,
                dgp=dgp, dge=dge, dwcb=dwcb, masks=masks, eye=eye)


def kernel(x, wq_w, wq_b, wkv_w, wkv_b, dwc_w, dwc_b, proj_w, proj_b,
           _want_results=False, **_unused):
    nc = _get_nc()
    consts = _host_consts(wq_w, wq_b, wkv_w, wkv_b, dwc_w, dwc_b, proj_w, proj_b)
    x = np.asarray(x, np.float32)
    in_maps = []
    for c in range(NCORES):
        m = dict(consts)
        m["x"] = np.ascontiguousarray(x[BL * c:BL * (c + 1)].reshape(T, DIM))
        in_maps.append(m)
    res = bass_utils.run_bass_kernel_spmd(nc, in_maps, core_ids=list(range(NCORES)))
    y = np.stack([res.results[c]["y"].reshape(BL, N, DIM) for c in range(NCORES)])
    y = y.reshape(B, N, DIM)
    if _want_results:
        return y, res
    return y


# revision 30
# speedup vs baseline: 1.0716x; 1.0328x over previous
"""Trainium2 Bass kernel for DiT focused-linear-attention block (nn_DiT_9259949490457).

Data-parallel over batch: 16 batches -> 8 NeuronCores, 2 batches/core, no collectives.

q and kv GEMMs run in fp8-e4m3 hi/lo split-precision using DoubleRow perf mode
(2 slice-products per 0.5-cycle/row instruction -> 14 DR units vs 18 bf16 units per
K=1152 contraction, with better-than-bf16 accuracy). The hi/lo fp8 cast of x is fused
into the transpose-psum evacuation (ACT writes hi, DVE subtract writes lo). The
depthwise 3x3 conv branch pairs (dy=-1,dy=+1) taps into DoubleRow fp8 matmuls via
custom overlapping access patterns.

The two local batches are phase-interleaved so PE always has matmul work while the
other batch's norm chain / einsum2-evacuation drains on ACT/DVE:
  b0:[K,V,vT,e1,Q,u6,dwc] b1:K b0:[norms,e2] b1:[V,vT,e1] b0:proj[0:6]
  b1:[Q,u6,dwc] b0:proj[6:8] b1:[norms,e2,proj]

Self-contained: hardcodes all shapes; host numpy pre-packs fp8 hi/lo weights
(scaled by 64; 1/64 folded into psum-evacuation activations).
"""

import numpy as np
import ml_dtypes
import bass_rust

import concourse.bacc as bacc
import concourse.mybir as mybir
import concourse.tile as tile
from concourse import bass_utils

F32 = mybir.dt.float32
BF16 = mybir.dt.bfloat16
FP8 = mybir.dt.float8e4
ALU = mybir.AluOpType
AF = mybir.ActivationFunctionType
AX = mybir.AxisListType
DR = mybir.MatmulPerfMode.DoubleRow

NCORES = 8
B, N, DIM = 16, 1024, 1152
H, KVH, HD = 12, 4, 96
BL = B // NCORES          # 2 local batches
T = BL * N                # 2048 local tokens
NK = DIM // 128           # 9 feature K-slices
TT = N // 128             # 8 token tiles per batch
CH = N // 512             # 2 free-dim chunks of 512 per batch
SW = 64.0                 # fp8 weight pre-scale (power of two)

_BF = ml_dtypes.bfloat16
_F8 = ml_dtypes.float8_e4m3fn


def _spanp(b):
    if b % 128 == 0:
        return 128
    if b % 64 == 0:
        return 64
    return 32


def _head_pieces(h):
    out = []
    rr = 0
    while rr < 96:
        gr = 96 * h + rr
        j, r0 = divmod(gr, 128)
        cnt = min(96 - rr, 128 - r0, _spanp(r0), _spanp(rr))
        out.append((j, r0, rr, cnt))
        rr += cnt
    return out


def _ins_dim(ap, stride, count):
    """Insert a free dim [stride, count] right after the partition dim."""
    dims = [list(d) for d in ap.ap]
    new = [dims[0], [stride, count]] + dims[1:]
    return bass_rust.AP(ap.tensor, ap.offset, new)


class _St:
    pass


def _build_kernel():
    nc = bacc.Bacc("TRN2", target_bir_lowering=False, debug=False,
                   enable_asserts=True, num_devices=NCORES)
    x_in = nc.dram_tensor("x", [T, DIM], F32, kind="ExternalInput").ap()
    wqfj_in = nc.dram_tensor("wqfj", [128, NK, 2, NK, 128], FP8, kind="ExternalInput").ap()
    wq8r_in = nc.dram_tensor("wq8r", [128, 2, DIM], FP8, kind="ExternalInput").ap()
    wkvfk_in = nc.dram_tensor("wkvfk", [128, 2, NK, 384], FP8, kind="ExternalInput").ap()
    wkvfv_in = nc.dram_tensor("wkvfv", [128, 2, NK, 384], FP8, kind="ExternalInput").ap()
    wkv8rk_in = nc.dram_tensor("wkv8rk", [128, 2, 384], FP8, kind="ExternalInput").ap()
    wkv8rv_in = nc.dram_tensor("wkv8rv", [128, 2, 384], FP8, kind="ExternalInput").ap()
    pwT_in = nc.dram_tensor("pwT", [DIM, DIM], BF16, kind="ExternalInput").ap()
    wqb_in = nc.dram_tensor("wqb", [128, NK], F32, kind="ExternalInput").ap()
    kvb64_in = nc.dram_tensor("kvb64", [1, 768], BF16, kind="ExternalInput").ap()
    pjb_bc_in = nc.dram_tensor("pjb_bc", [1, DIM], BF16, kind="ExternalInput").ap()
    dgp_in = nc.dram_tensor("dgp", [96, 2, KVH, 3, 96], FP8, kind="ExternalInput").ap()
    dge_in = nc.dram_tensor("dge", [96, KVH, 9, 96], FP8, kind="ExternalInput").ap()
    dwcb_in = nc.dram_tensor("dwcb", [96, KVH], F32, kind="ExternalInput").ap()
    masks_in = nc.dram_tensor("masks", [128, NK, H], BF16, kind="ExternalInput").ap()
    eye_in = nc.dram_tensor("eye", [128, 128], BF16, kind="ExternalInput").ap()
    y_out = nc.dram_tensor("y", [T, DIM], F32, kind="ExternalOutput").ap()

    from contextlib import ExitStack
    with tile.TileContext(nc) as tc, ExitStack() as stack:
        cpool = stack.enter_context(tc.tile_pool(name="const", bufs=1))
        dpool = stack.enter_context(tc.tile_pool(name="dram", bufs=1, space="DRAM"))

        # ---- consts / weights ----
        eye = cpool.tile([128, 128], BF16, name="eye")
        wkvfk = cpool.tile([128, 2, NK, 384], FP8, name="wkvfk")
        wkvfv = cpool.tile([128, 2, NK, 384], FP8, name="wkvfv")
        wkv8rk = cpool.tile([128, 2, 384], FP8, name="wkv8rk")
        wkv8rv = cpool.tile([128, 2, 384], FP8, name="wkv8rv")
        kvb64 = cpool.tile([1, 768], BF16, name="kvb64")
        wqfj = cpool.tile([128, NK, 2, NK, 128], FP8, name="wqfj")
        wq8r = cpool.tile([128, 2, DIM], FP8, name="wq8r")
        wqb = cpool.tile([128, NK], F32, name="wqb")
        PWT = [cpool.tile([128, DIM], BF16, name=f"PWT{k}") for k in range(NK)]
        pjb_bc = cpool.tile([1, DIM], BF16, name="pjb_bc")
        dgp = cpool.tile([96, 2, KVH, 3, 96], FP8, name="dgp")
        dge = cpool.tile([96, KVH, 9, 96], FP8, name="dge")
        dwcb = cpool.tile([96, KVH], F32, name="dwcb")
        masks = cpool.tile([128, NK, H], BF16, name="masks")
        ones_r = cpool.tile([1, 128], BF16, name="ones_r")
        ones_c = cpool.tile([128, 1], BF16, name="ones_c")
        sqwarm = cpool.tile([1, 8], F32, name="sqwarm")

        vpad = dpool.tile([BL, N, KVH, 128], BF16, name="vpad")

        # ---- pools ----
        xpool = stack.enter_context(tc.tile_pool(name="xf", bufs=1))
        XF = xpool.tile([128, 2, NK, T], FP8, name="XF")
        wp = stack.enter_context(tc.tile_pool(name="work", bufs=1))
        pmm = stack.enter_context(tc.tile_pool(name="pmm", bufs=1, space="PSUM"))
        pa = stack.enter_context(tc.tile_pool(name="pa", bufs=1, space="PSUM"))

        # ---- prologue machinery: load x, transpose on PE, evacuate as fp8
        # hi/lo into XF. Stages are interleaved with batch-0 K/V tiles so the
        # ACT/DVE evacuation queue never runs ahead of the GEMM consumers.
        prep = stack.enter_context(tc.tile_pool(name="prep", bufs=3))

        def prep_stage(i):
            stage = prep.tile([128, DIM], BF16, name="stage", tag="stage")
            nc.gpsimd.dma_start(out=stage[:],
                                in_=x_in[128 * i:128 * (i + 1), :])
            if i == 0:
                nc.sync.dma_start(out=eye[:], in_=eye_in[:])
                nc.vector.memset(ones_r[:], 1.0)
                nc.vector.memset(ones_c[:], 1.0)
                nc.vector.memset(sqwarm[:], 1.0)
                # warm the Sqrt activation table off the critical path
                nc.scalar.activation(sqwarm[:], sqwarm[:], AF.Sqrt)
            elif i == 1:
                nc.sync.dma_start(out=wkvfk[:], in_=wkvfk_in[:])
                nc.sync.dma_start(out=wkv8rk[:], in_=wkv8rk_in[:])
                nc.sync.dma_start(out=kvb64[:], in_=kvb64_in[:])
            elif i == 3:
                nc.sync.dma_start(out=wkvfv[:], in_=wkvfv_in[:])
                nc.sync.dma_start(out=wkv8rv[:], in_=wkv8rv_in[:])
            elif i == 5:
                for j in range(NK):
                    nc.sync.dma_start(out=wqfj[:, j], in_=wqfj_in[:, j])
                nc.sync.dma_start(out=wq8r[:], in_=wq8r_in[:])
                nc.sync.dma_start(out=wqb[:], in_=wqb_in[:])
            elif i == 9:
                for k in range(NK):
                    nc.sync.dma_start(out=PWT[k][:],
                                      in_=pwT_in[128 * k:128 * (k + 1), :])
                nc.sync.dma_start(out=pjb_bc[:], in_=pjb_bc_in[:])
            elif i == 11:
                nc.sync.dma_start(out=dgp[:], in_=dgp_in[:])
                nc.sync.dma_start(out=dge[:], in_=dge_in[:])
                nc.sync.dma_start(out=dwcb[:], in_=dwcb_in[:])
                nc.sync.dma_start(out=masks[:], in_=masks_in[:])
            pt_a = pa.tile([128, 1024], BF16, name="pt_a", tag="pa", bufs=2)
            for k in range(8):
                nc.tensor.transpose(pt_a[:, 128 * k:128 * (k + 1)],
                                    stage[:, 128 * k:128 * (k + 1)], eye[:])
            pt_b = pa.tile([128, 128], BF16, name="pt_b", tag="pa", bufs=2)
            nc.tensor.transpose(pt_b[:], stage[:, 1024:1152], eye[:])
            cs = slice(128 * i, 128 * (i + 1))
            pt3 = pt_a[:].rearrange("p (k c) -> p k c", c=128)
            # hi casts (ACT), slices 0-7 slot1, slice 8 slot0
            nc.scalar.activation(XF[:, 1, 0:8, cs], pt3, AF.Copy)
            nc.scalar.activation(XF[:, 0, 8, cs], pt_b[:], AF.Copy)
            # lo = psum - hi (DVE), slices 0-7 slot0, slice 8 slot1
            nc.vector.tensor_tensor(out=XF[:, 0, 0:8, cs], in0=pt3,
                                    in1=XF[:, 1, 0:8, cs], op=ALU.subtract)
            nc.vector.tensor_tensor(out=XF[:, 1, 8, cs], in0=pt_b[:],
                                    in1=XF[:, 0, 8, cs], op=ALU.subtract)

        cn = _St()
        cn.wqfj, cn.wq8r, cn.wqb = wqfj, wq8r, wqb
        cn.wkvfk, cn.wkvfv, cn.wkv8rk, cn.wkv8rv = wkvfk, wkvfv, wkv8rk, wkv8rv
        cn.kvb64, cn.PWT, cn.pjb_bc = kvb64, PWT, pjb_bc
        cn.dgp, cn.dge, cn.dwcb, cn.masks = dgp, dge, dwcb, masks
        cn.ones_r, cn.ones_c, cn.XF, cn.vpad, cn.y_out = ones_r, ones_c, XF, vpad, y_out

        p0 = _phases(nc, 0, wp, pmm, pa, cn)
        p1 = _phases(nc, 1, wp, pmm, pa, cn)
        # x stages feed b0's K/V tiles just-in-time
        prep_stage(0)
        prep_stage(1)
        p0["k_begin"]()
        for t in range(TT):
            if t + 2 < 8:
                prep_stage(t + 2)
            p0["k_tile"](t)
        for t in range(TT):
            prep_stage(8 + t)
            p0["v_tile"](t)
        p0["v_end"]()
        p0["vt"](); p0["e1"](); p0["q"](); p0["u6"](); p0["dwc"]()
        p1["k_begin"]()
        for t in range(TT):
            p1["k_tile"](t)
        p0["norms"](); p0["e2"]()
        for t in range(TT):
            p1["v_tile"](t)
        p1["v_end"]()
        p1["vt"](); p1["e1"]()
        p0["proj"](0, 6)
        p1["q"](); p1["u6"](); p1["dwc"]()
        p0["proj"](6, 8)
        p1["norms"](); p1["e2"](); p1["proj"](0, 8)

    nc.compile()
    return nc


def _phases(nc, b, wp, pmm, pa, cn):
    st = _St()
    XF = cn.XF

    def emit_kv_half(t, vhalf, out_pk):
        t0 = b * N + 128 * t
        wf = cn.wkvfv if vhalf else cn.wkvfk
        w8 = cn.wkv8rv if vhalf else cn.wkv8rk
        for ci, cc in ((0, 0), (192, 192)):
            dst = out_pk[:, ci:ci + 192]
            for a in range(4):
                nc.tensor.matmul(dst, XF[:, 1, 2 * a:2 * a + 2, t0:t0 + 128],
                                 wf[:, 0, 2 * a:2 * a + 2, cc:cc + 192],
                                 start=(a == 0), stop=False, perf_mode=DR)
            nc.tensor.matmul(dst, XF[:, :, 8, t0:t0 + 128],
                             wf[:, :, 8, cc:cc + 192],
                             start=False, stop=False, perf_mode=DR)
            for k in range(8):
                nc.tensor.matmul(dst, XF[:, :, k, t0:t0 + 128],
                                 wf[:, :, k, cc:cc + 192],
                                 start=False, stop=False, perf_mode=DR)
            nc.tensor.matmul(dst, XF[:, :, 8, t0:t0 + 128],
                             w8[:, :, cc:cc + 192],
                             start=False, stop=False, perf_mode=DR)
            bc = 384 * vhalf + cc
            nc.tensor.matmul(dst, cn.ones_r[:], cn.kvb64[:, bc:bc + 192],
                             start=False, stop=True)

    def ph_k_begin():
        st.k3 = [wp.tile([128, 384], BF16, name=f"k3_{t}", tag=f"k3_{t}")
                 for t in range(TT)]
        st.vv = [wp.tile([128, 384], BF16, name=f"v_{t}", tag=f"v_{t}")
                 for t in range(TT)]
        st.acc2k = wp.tile([128, KVH * TT], F32, name="acc2k", tag="acc2k", bufs=2)
        st.uk2s = []

    def ph_k_tile(t):
        pk = pmm.tile([128, 512], F32, name="pk", tag="pmm", bufs=3)
        emit_kv_half(t, 0, pk)
        if t == 0:
            st.acc1kr = pmm.tile([1, 384], F32, name="acc1kr", tag="pnorm",
                                 bufs=1)
        if t >= 2:
            # row-accumulate acc1k at a 2-tile lag so PE never waits on ACT
            nc.tensor.matmul(st.acc1kr[:], cn.ones_c[:], st.uk2s[t - 2][:],
                             start=(t == 2), stop=False)
        uk = wp.tile([128, 384], BF16, name="uk", tag="uk", bufs=2)
        nc.scalar.activation(uk[:], pk[:, 0:384], AF.Relu, scale=1.0 / SW)
        uk2 = wp.tile([128, 384], BF16, name="uk2", tag="uk2", bufs=3)
        st.uk2s.append(uk2)
        nc.scalar.activation(uk2[:], uk[:], AF.Square)
        nc.vector.tensor_mul(st.k3[t][:], uk2[:], uk[:])
        uk6 = wp.tile([128, 384], BF16, name="uk6", tag="uk6", bufs=2)
        nc.vector.tensor_mul(uk6[:], st.k3[t][:], st.k3[t][:])
        for g in range(KVH):
            nc.vector.tensor_reduce(st.acc2k[:, g * TT + t:g * TT + t + 1],
                                    uk6[:, 96 * g:96 * (g + 1)],
                                    axis=AX.X, op=ALU.add)

    def ph_v_tile(t):
        pv = pmm.tile([128, 512], F32, name="pv", tag="pmm", bufs=3)
        if t < 2:
            # flush the lagged acc1k row-accumulation
            nc.tensor.matmul(st.acc1kr[:], cn.ones_c[:],
                             st.uk2s[TT - 2 + t][:],
                             start=False, stop=(t == 1))
        emit_kv_half(t, 1, pv)
        nc.vector.tensor_scalar_mul(st.vv[t][:], pv[:, 0:384], 1.0 / SW)
        nc.sync.dma_start(
            out=cn.vpad[b, 128 * t:128 * (t + 1), :, 0:96],
            in_=st.vv[t][:].rearrange("p (k d) -> p k d", k=KVH))

    def ph_v_end():
        # k-side acc1 group sums: free the pnorm psum row early
        st.kred1 = wp.tile([1, KVH], F32, name="kred1", tag="kred1", bufs=2)
        nc.vector.tensor_reduce(st.kred1[:],
                                st.acc1kr[:].rearrange("a (k d) -> a k d", k=KVH),
                                axis=AX.X, op=ALU.add)

    def ph_vt():
        st.vT8 = []
        for g in range(KVH):
            vT = wp.tile([128, N], BF16, name="vTd", tag="vTd", bufs=2)
            nc.sync.dma_start(out=vT[:], in_=cn.vpad[b, :, g, :], transpose=True)
            v8 = wp.tile([96, N], FP8, name="v8", tag=f"v8_{g}")
            if g % 2 == 0:
                nc.scalar.activation(v8[:], vT[0:96, :], AF.Copy)
            else:
                nc.vector.tensor_copy(v8[:], vT[0:96, :])
            st.vT8.append(v8)

    def ph_e1():
        st.kvu = [wp.tile([96, 96], BF16, name=f"kvu_{g}", tag=f"kvu_{g}")
                  for g in range(KVH)]
        for g in range(KVH):
            pk_t = pa.tile([96, 96], F32, name="pkvt", tag="pa", bufs=2)
            for t in range(TT):
                nc.tensor.matmul(pk_t[:], st.k3[t][:, 96 * g:96 * (g + 1)],
                                 st.vv[t][:, 96 * g:96 * (g + 1)],
                                 start=(t == 0), stop=(t == TT - 1))
            nc.vector.tensor_copy(st.kvu[g][:], pk_t[:])

    def ph_q():
        st.acc1q = wp.tile([128, NK * CH], F32, name="acc1q", tag="acc1q")
        st.acc2q = wp.tile([128, NK * CH], F32, name="acc2q", tag="acc2q")
        st.q3 = [wp.tile([128, N], BF16, name=f"q3_{j}", tag=f"q3_{j}")
                 for j in range(NK)]
        wqfj, wq8r = cn.wqfj, cn.wq8r
        for c2 in range(CH):
            for j in range(NK):
                pq = pmm.tile([128, 512], F32, name="pq", tag="pmm", bufs=3)
                for sub in range(2):
                    t0 = b * N + 512 * c2 + 256 * sub
                    dst = pq[:, 256 * sub:256 * (sub + 1)]
                    for a in range(4):
                        nc.tensor.matmul(dst, wqfj[:, j, 0, 2 * a:2 * a + 2, :],
                                         XF[:, 1, 2 * a:2 * a + 2, t0:t0 + 256],
                                         start=(a == 0), stop=False, perf_mode=DR)
                    nc.tensor.matmul(dst, wqfj[:, j, :, 8, :],
                                     XF[:, :, 8, t0:t0 + 256],
                                     start=False, stop=False, perf_mode=DR)
                    for k in range(8):
                        nc.tensor.matmul(dst, wqfj[:, j, :, k, :],
                                         XF[:, :, k, t0:t0 + 256],
                                         start=False, stop=False, perf_mode=DR)
                    nc.tensor.matmul(dst, wq8r[:, :, 128 * j:128 * (j + 1)],
                                     XF[:, :, 8, t0:t0 + 256],
                                     start=False, stop=(sub == 1), perf_mode=DR)
                u = wp.tile([128, 512], BF16, name="u", tag="u", bufs=2)
                nc.scalar.activation(u[:], pq[:], AF.Relu, scale=1.0 / SW,
                                     bias=cn.wqb[:, j:j + 1])
                u2 = wp.tile([128, 512], BF16, name="u2", tag="u2", bufs=2)
                col = j * CH + c2
                nc.scalar.activation(u2[:], u[:], AF.Square,
                                     accum_out=st.acc1q[:, col:col + 1])
                nc.vector.tensor_mul(st.q3[j][:, 512 * c2:512 * (c2 + 1)],
                                     u2[:], u[:])

    def ph_u6():
        # deferred acc2q: runs after the Q GEMMs (overlapping the dwconv
        # window) instead of blocking psum evacuation; split ACT/DVE
        for c2 in range(CH):
            for j in range(NK):
                col = j * CH + c2
                q3s = st.q3[j][:, 512 * c2:512 * (c2 + 1)]
                u6 = wp.tile([128, 512], BF16, name="u6", tag="u6", bufs=2)
                if (j + c2) % 2 == 0:
                    nc.scalar.activation(u6[:], q3s, AF.Square,
                                         accum_out=st.acc2q[:, col:col + 1])
                else:
                    nc.vector.tensor_mul(u6[:], q3s, q3s)
                    nc.vector.tensor_reduce(st.acc2q[:, col:col + 1], u6[:],
                                            axis=AX.X, op=ALU.add)

    def ph_dwc():
        st.vdwc = [wp.tile([96, N], BF16, name=f"vdwc_{g}", tag=f"vdwc_{g}")
                   for g in range(KVH)]
        for g in range(KVH):
            v3 = st.vT8[g][:].rearrange("p (y x) -> p y x", y=32)
            for hf in range(2):
                pd = pmm.tile([96, 512], F32, name="pd", tag="pdw", bufs=2)
                p3 = pd[:].rearrange("p (y x) -> p y x", y=16)
                mms = []
                for dxi, dx in enumerate((-1, 0, 1)):
                    x0, x1 = max(0, -dx), 32 - max(0, dx)
                    mms.append((cn.dge[:, g, 3 + dxi, :],
                                v3[0:96, 16 * hf:16 * hf + 16, x0 + dx:x1 + dx],
                                p3[:, 0:16, x0:x1], None))
                ya0 = max(1, 16 * hf)
                ya1 = min(31, 16 * hf + 16)
                for dxi, dx in enumerate((-1, 0, 1)):
                    x0, x1 = max(0, -dx), 32 - max(0, dx)
                    base = v3[0:96, ya0 - 1:ya1 - 1, x0 + dx:x1 + dx]
                    rhs = _ins_dim(base, 64, 2)
                    mms.append((cn.dgp[:, :, g, dxi, :], rhs,
                                p3[:, ya0 - 16 * hf:ya1 - 16 * hf, x0:x1], DR))
                for dxi, dx in enumerate((-1, 0, 1)):
                    x0, x1 = max(0, -dx), 32 - max(0, dx)
                    if hf == 0:  # y=0, tap dy=+1
                        mms.append((cn.dge[:, g, 6 + dxi, :],
                                    v3[0:96, 1:2, x0 + dx:x1 + dx],
                                    p3[:, 0:1, x0:x1], None))
                    else:        # y=31, tap dy=-1
                        mms.append((cn.dge[:, g, dxi, :],
                                    v3[0:96, 30:31, x0 + dx:x1 + dx],
                                    p3[:, 15:16, x0:x1], None))
                for mi, (lhsT, rhs, out, pm) in enumerate(mms):
                    nc.tensor.matmul(out, lhsT, rhs, start=(mi == 0),
                                     stop=(mi == len(mms) - 1), perf_mode=pm)
                nc.scalar.activation(st.vdwc[g][:, 512 * hf:512 * (hf + 1)],
                                     pd[:], AF.Identity, scale=1.0 / SW,
                                     bias=cn.dwcb[:, g:g + 1])

    def ph_norms():
        sq_rows = []
        for ai, acc in enumerate((st.acc1q, st.acc2q)):
            accs = wp.tile([128, NK], F32, name="accs", tag="accs", bufs=2)
            av = acc[:, 0:NK * CH].rearrange("p (j c) -> p j c", c=CH)
            nc.vector.tensor_add(accs[:], av[:, :, 0], av[:, :, 1])
            accsb = wp.tile([128, NK], BF16, name="accsb", tag="accsb", bufs=2)
            nc.vector.tensor_copy(accsb[:], accs[:])
            psn = pa.tile([1, H], F32, name="psn", tag="pa", bufs=2)
            for j in range(NK):
                nc.tensor.matmul(psn[:], accsb[:, j:j + 1], cn.masks[:, j, :],
                                 start=(j == 0), stop=(j == NK - 1))
            srow = wp.tile([1, H], F32, name="srow", tag="srow", bufs=4)
            nc.vector.tensor_copy(srow[:], psn[:])
            sq_rows.append(srow)
        acc2kb = wp.tile([128, KVH * TT], BF16, name="acc2kb", tag="acc2kb",
                         bufs=2)
        nc.vector.tensor_copy(acc2kb[:], st.acc2k[:])
        psk = pa.tile([1, KVH * TT], F32, name="psk", tag="pa", bufs=2)
        nc.tensor.matmul(psk[:], cn.ones_c[:], acc2kb[:], start=True, stop=True)
        krow = wp.tile([1, KVH * TT], F32, name="krow", tag="krow", bufs=2)
        nc.vector.tensor_copy(krow[:], psk[:])
        kred2 = wp.tile([1, KVH], F32, name="kred2", tag="kred2", bufs=2)
        nc.vector.tensor_reduce(kred2[:],
                                krow[:].rearrange("a (k t) -> a k t", k=KVH),
                                axis=AX.X, op=ALU.add)
        sk_rows = [st.kred1, kred2]

        def _f_row(s1, s2, width, tagp):
            se = wp.tile([1, width], F32, name="se", tag=f"se{tagp}", bufs=2)
            nc.vector.tensor_scalar_add(se[:], s2[:], 1e-30)
            rc = wp.tile([1, width], F32, name="rc", tag=f"rc{tagp}", bufs=2)
            nc.vector.reciprocal(rc[:], se[:])
            rt = wp.tile([1, width], F32, name="rt", tag=f"rt{tagp}", bufs=2)
            nc.vector.tensor_mul(rt[:], s1[:], rc[:])
            fr = wp.tile([1, width], F32, name="fr", tag=f"fr{tagp}", bufs=2)
            nc.scalar.activation(fr[:], rt[:], AF.Sqrt)
            return fr

        fq = _f_row(sq_rows[0], sq_rows[1], H, "q")
        fk = _f_row(sk_rows[0], sk_rows[1], KVH, "k")
        fk12 = wp.tile([1, H], F32, name="fk12", tag="fk12", bufs=2)
        for g in range(3):
            nc.vector.tensor_copy(fk12[:, 4 * g:4 * (g + 1)], fk[:])
        grow = wp.tile([1, H], F32, name="grow", tag="grow", bufs=2)
        nc.vector.tensor_mul(grow[:], fq[:], fk12[:])
        gb = wp.tile([96, H], F32, name="gb", tag="gb", bufs=2)
        nc.gpsimd.partition_broadcast(gb[:], grow[:], channels=96)
        st.kvp = [wp.tile([96, 96], BF16, name=f"kvp_{h}", tag=f"kvp_{h}")
                  for h in range(H)]
        for h in range(H):
            nc.vector.tensor_scalar_mul(st.kvp[h][:], st.kvu[h % KVH][:],
                                        gb[:, h:h + 1])

    def ph_e2():
        st.OT = [wp.tile([128, N], BF16, name=f"OT_{j}", tag=f"OT_{j}")
                 for j in range(NK)]
        for c2 in range(CH):
            for h in range(H):
                pieces = _head_pieces(h)
                if len(pieces) == 1:
                    j0, r00, _, _ = pieces[0]
                    rhs = st.q3[j0][r00:r00 + 96, 512 * c2:512 * (c2 + 1)]
                else:
                    qh = wp.tile([96, 512], BF16, name="qh", tag="qh", bufs=4)
                    for pi, (j, r0, rr, cnt) in enumerate(pieces):
                        src_ap = st.q3[j][r0:r0 + cnt, 512 * c2:512 * (c2 + 1)]
                        if (h + pi) % 2 == 0:
                            nc.vector.tensor_copy(qh[rr:rr + cnt, :], src_ap)
                        else:
                            nc.scalar.copy(qh[rr:rr + cnt, :], src_ap)
                    rhs = qh[:]
                pa_t = pa.tile([96, 512], F32, name="pat", tag="pa", bufs=2)
                nc.tensor.matmul(pa_t[:], st.kvp[h][:], rhs, start=True,
                                 stop=True)
                if len(pieces) == 1:
                    j0, r00, _, _ = pieces[0]
                    nc.vector.tensor_tensor(
                        out=st.OT[j0][r00:r00 + 96, 512 * c2:512 * (c2 + 1)],
                        in0=pa_t[:],
                        in1=st.vdwc[h % KVH][:, 512 * c2:512 * (c2 + 1)],
                        op=ALU.add)
                else:
                    pac = wp.tile([96, 512], BF16, name="pac", tag="pac", bufs=4)
                    nc.scalar.copy(pac[:], pa_t[:])
                    for (j, r0, rr, cnt) in pieces:
                        nc.vector.tensor_tensor(
                            out=st.OT[j][r0:r0 + cnt, 512 * c2:512 * (c2 + 1)],
                            in0=pac[rr:rr + cnt, :],
                            in1=st.vdwc[h % KVH][rr:rr + cnt,
                                                 512 * c2:512 * (c2 + 1)],
                            op=ALU.add)

    def ph_proj(ta, tb):
        for t in range(ta, tb):
            for oc in range(3):
                py = pmm.tile([128, 384], F32, name="py", tag="pdw", bufs=2)
                for j in range(NK):
                    nc.tensor.matmul(py[:], st.OT[j][:, 128 * t:128 * (t + 1)],
                                     cn.PWT[j][:, 384 * oc:384 * (oc + 1)],
                                     start=(j == 0), stop=False)
                nc.tensor.matmul(py[:], cn.ones_r[:],
                                 cn.pjb_bc[:, 384 * oc:384 * (oc + 1)],
                                 start=False, stop=True)
                ysb = wp.tile([128, 384], F32, name="ysb", tag="ysb", bufs=3)
                if (t + oc) % 2 == 0:
                    nc.scalar.copy(ysb[:], py[:])
                else:
                    nc.vector.tensor_copy(ysb[:], py[:])
                t0 = b * N + 128 * t
                nc.sync.dma_start(out=cn.y_out[t0:t0 + 128,
                                               384 * oc:384 * (oc + 1)],
                                  in_=ysb[:])

    return dict(k_begin=ph_k_begin, k_tile=ph_k_tile, v_tile=ph_v_tile,
                v_end=ph_v_end, vt=ph_vt, e1=ph_e1, q=ph_q, u6=ph_u6,
                dwc=ph_dwc, norms=ph_norms, e2=ph_e2, proj=ph_proj)


_NC_CACHE = None


def _get_nc():
    global _NC_CACHE
    if _NC_CACHE is None:
        _NC_CACHE = _build_kernel()
    return _NC_CACHE


def _hi_lo(a):
    hi = a.astype(_F8)
    lo = (a - hi.astype(np.float32)).astype(_F8)
    return hi, lo


def _host_consts(wq_w, wq_b, wkv_w, wkv_b, dwc_w, dwc_b, proj_w, proj_b):
    wqT = np.ascontiguousarray(np.asarray(wq_w, np.float32).T) * SW      # [in, out]
    wkvT = np.ascontiguousarray(np.asarray(wkv_w, np.float32).T) * SW    # [in, 768]
    qhi, qlo = _hi_lo(wqT)
    khi, klo = _hi_lo(wkvT)

    # wqfj: [128, j, slot(hi,lo), k, 128]
    wqfj = np.zeros((128, NK, 2, NK, 128), _F8)
    for k in range(NK):
        for j in range(NK):
            wqfj[:, j, 0, k, :] = qhi[128 * k:128 * (k + 1), 128 * j:128 * (j + 1)]
            wqfj[:, j, 1, k, :] = qlo[128 * k:128 * (k + 1), 128 * j:128 * (j + 1)]
    wq8r = np.zeros((128, 2, DIM), _F8)
    wq8r[:, 0, :] = qlo[128 * 8:, :]
    wq8r[:, 1, :] = qhi[128 * 8:, :]

    wkvf = np.zeros((128, 2, NK, 768), _F8)
    for k in range(NK):
        wkvf[:, 0, k, :] = khi[128 * k:128 * (k + 1), :]
        wkvf[:, 1, k, :] = klo[128 * k:128 * (k + 1), :]
    wkv8r = np.zeros((128, 2, 768), _F8)
    wkv8r[:, 0, :] = klo[128 * 8:, :]
    wkv8r[:, 1, :] = khi[128 * 8:, :]
    wkvfk = np.ascontiguousarray(wkvf[:, :, :, 0:384])
    wkvfv = np.ascontiguousarray(wkvf[:, :, :, 384:768])
    wkv8rk = np.ascontiguousarray(wkv8r[:, :, 0:384])
    wkv8rv = np.ascontiguousarray(wkv8r[:, :, 384:768])

    pwT = np.ascontiguousarray(np.asarray(proj_w, np.float32).T).astype(_BF)
    wqb = np.ascontiguousarray(np.asarray(wq_b, np.float32).reshape(NK, 128).T)
    kvb64 = (np.asarray(wkv_b, np.float32).reshape(1, 768) * SW).astype(_BF)
    pjb_bc = np.asarray(proj_b, np.float32).reshape(1, DIM).astype(_BF)

    dw = np.asarray(dwc_w, np.float32).reshape(KVH, 96, 9) * SW  # [g, d, tap]
    dgp = np.zeros((96, 2, KVH, 3, 96), np.float32)
    dge = np.zeros((96, KVH, 9, 96), np.float32)
    for d in range(96):
        for dxi in range(3):
            dgp[d, 0, :, dxi, d] = dw[:, d, 0 + dxi]       # dy=-1 taps 0,1,2
            dgp[d, 1, :, dxi, d] = dw[:, d, 6 + dxi]       # dy=+1 taps 6,7,8
        for ti in range(9):
            dge[d, :, ti, d] = dw[:, d, ti]
    dgp = dgp.astype(_F8)
    dge = dge.astype(_F8)
    dwcb = np.ascontiguousarray(np.asarray(dwc_b, np.float32).reshape(KVH, 96).T)

    mk = np.zeros((128, NK, H), np.float32)
    for j in range(NK):
        for p in range(128):
            f = 128 * j + p
            mk[p, j, f // 96] = 1.0
    masks = mk.astype(_BF)
    eye = np.eye(128, dtype=np.float32).astype(_BF)
    return dict(wqfj=wqfj, wq8r=wq8r, wkvfk=wkvfk, wkvfv=wkvfv, wkv8rk=wkv8rk,
                wkv8rv=wkv8rv, pwT=pwT, wqb=wqb, kvb64=kvb64, pjb_bc=pjb_bc,
                dgp=dgp, dge=dge, dwcb=dwcb, masks=masks, eye=eye)


def kernel(x, wq_w, wq_b, wkv_w, wkv_b, dwc_w, dwc_b, proj_w, proj_b,
           _want_results=False, **_unused):
    nc = _get_nc()
    consts = _host_consts(wq_w, wq_b, wkv_w, wkv_b, dwc_w, dwc_b, proj_w, proj_b)
    x = np.asarray(x, np.float32)
    in_maps = []
    for c in range(NCORES):
        m = dict(consts)
        m["x"] = np.ascontiguousarray(x[BL * c:BL * (c + 1)].reshape(T, DIM))
        in_maps.append(m)
    res = bass_utils.run_bass_kernel_spmd(nc, in_maps, core_ids=list(range(NCORES)))
    y = np.stack([res.results[c]["y"].reshape(BL, N, DIM) for c in range(NCORES)])
    y = y.reshape(B, N, DIM)
    if _want_results:
        return y, res
    return y


# revision 35
# speedup vs baseline: 1.0870x; 1.0144x over previous
"""Trainium2 Bass kernel for DiT focused-linear-attention block (nn_DiT_9259949490457).

Data-parallel over batch: 16 batches -> 8 NeuronCores, 2 batches/core, no collectives.

q and kv GEMMs run in fp8-e4m3 hi/lo split-precision using DoubleRow perf mode
(2 slice-products per 0.5-cycle/row instruction -> 14 DR units vs 18 bf16 units per
K=1152 contraction, with better-than-bf16 accuracy). The hi/lo fp8 cast of x is fused
into the transpose-psum evacuation (ACT writes hi, DVE subtract writes lo). The
depthwise 3x3 conv branch pairs (dy=-1,dy=+1) taps into DoubleRow fp8 matmuls via
custom overlapping access patterns.

The two local batches are phase-interleaved so PE always has matmul work while the
other batch's norm chain / einsum2-evacuation drains on ACT/DVE:
  b0:[K,V,vT,e1,Q,u6,dwc] b1:K b0:[norms,e2] b1:[V,vT,e1] b0:proj[0:6]
  b1:[Q,u6,dwc] b0:proj[6:8] b1:[norms,e2,proj]

Self-contained: hardcodes all shapes; host numpy pre-packs fp8 hi/lo weights
(scaled by 64; 1/64 folded into psum-evacuation activations).
"""

import numpy as np
import ml_dtypes
import bass_rust

import concourse.bacc as bacc
import concourse.mybir as mybir
import concourse.tile as tile
from concourse import bass_utils

F32 = mybir.dt.float32
BF16 = mybir.dt.bfloat16
FP8 = mybir.dt.float8e4
ALU = mybir.AluOpType
AF = mybir.ActivationFunctionType
AX = mybir.AxisListType
DR = mybir.MatmulPerfMode.DoubleRow

NCORES = 8
B, N, DIM = 16, 1024, 1152
H, KVH, HD = 12, 4, 96
BL = B // NCORES          # 2 local batches
T = BL * N                # 2048 local tokens
NK = DIM // 128           # 9 feature K-slices
TT = N // 128             # 8 token tiles per batch
CH = N // 512             # 2 free-dim chunks of 512 per batch
SW = 64.0                 # fp8 weight pre-scale (power of two)

_BF = ml_dtypes.bfloat16
_F8 = ml_dtypes.float8_e4m3fn


def _spanp(b):
    if b % 128 == 0:
        return 128
    if b % 64 == 0:
        return 64
    return 32


def _head_pieces(h):
    out = []
    rr = 0
    while rr < 96:
        gr = 96 * h + rr
        j, r0 = divmod(gr, 128)
        cnt = min(96 - rr, 128 - r0, _spanp(r0), _spanp(rr))
        out.append((j, r0, rr, cnt))
        rr += cnt
    return out


def _ins_dim(ap, stride, count):
    """Insert a free dim [stride, count] right after the partition dim."""
    dims = [list(d) for d in ap.ap]
    new = [dims[0], [stride, count]] + dims[1:]
    return bass_rust.AP(ap.tensor, ap.offset, new)


class _St:
    pass


def _build_kernel():
    nc = bacc.Bacc("TRN2", target_bir_lowering=False, debug=False,
                   enable_asserts=True, num_devices=NCORES)
    x_in = nc.dram_tensor("x", [T, DIM], F32, kind="ExternalInput").ap()
    wqfj_in = nc.dram_tensor("wqfj", [128, NK, 2, NK, 128], FP8, kind="ExternalInput").ap()
    wq8r_in = nc.dram_tensor("wq8r", [128, 2, DIM], FP8, kind="ExternalInput").ap()
    wkvfk_in = nc.dram_tensor("wkvfk", [128, 2, NK, 384], FP8, kind="ExternalInput").ap()
    wkvfv_in = nc.dram_tensor("wkvfv", [128, 2, NK, 384], FP8, kind="ExternalInput").ap()
    wkv8rk_in = nc.dram_tensor("wkv8rk", [128, 2, 384], FP8, kind="ExternalInput").ap()
    wkv8rv_in = nc.dram_tensor("wkv8rv", [128, 2, 384], FP8, kind="ExternalInput").ap()
    pwT_in = nc.dram_tensor("pwT", [DIM, DIM], BF16, kind="ExternalInput").ap()
    wqb_in = nc.dram_tensor("wqb", [128, NK], F32, kind="ExternalInput").ap()
    kvb64_in = nc.dram_tensor("kvb64", [1, 768], BF16, kind="ExternalInput").ap()
    pjb_bc_in = nc.dram_tensor("pjb_bc", [1, DIM], BF16, kind="ExternalInput").ap()
    dgp_in = nc.dram_tensor("dgp", [96, 2, KVH, 3, 96], FP8, kind="ExternalInput").ap()
    dge_in = nc.dram_tensor("dge", [96, KVH, 9, 96], FP8, kind="ExternalInput").ap()
    dwcb_in = nc.dram_tensor("dwcb", [96, KVH], F32, kind="ExternalInput").ap()
    masks_in = nc.dram_tensor("masks", [128, NK, H], BF16, kind="ExternalInput").ap()
    eye_in = nc.dram_tensor("eye", [128, 128], BF16, kind="ExternalInput").ap()
    y_out = nc.dram_tensor("y", [T, DIM], F32, kind="ExternalOutput").ap()

    from contextlib import ExitStack
    with tile.TileContext(nc) as tc, ExitStack() as stack:
        cpool = stack.enter_context(tc.tile_pool(name="const", bufs=1))
        dpool = stack.enter_context(tc.tile_pool(name="dram", bufs=1, space="DRAM"))

        # ---- consts / weights ----
        eye = cpool.tile([128, 128], BF16, name="eye")
        wkvfk = cpool.tile([128, 2, NK, 384], FP8, name="wkvfk")
        wkvfv = cpool.tile([128, 2, NK, 384], FP8, name="wkvfv")
        wkv8rk = cpool.tile([128, 2, 384], FP8, name="wkv8rk")
        wkv8rv = cpool.tile([128, 2, 384], FP8, name="wkv8rv")
        kvb64 = cpool.tile([1, 768], BF16, name="kvb64")
        wqfj = cpool.tile([128, NK, 2, NK, 128], FP8, name="wqfj")
        wq8r = cpool.tile([128, 2, DIM], FP8, name="wq8r")
        wqb = cpool.tile([128, NK], F32, name="wqb")
        PWT = [cpool.tile([128, DIM], BF16, name=f"PWT{k}") for k in range(NK)]
        pjb_bc = cpool.tile([1, DIM], BF16, name="pjb_bc")
        dgp = cpool.tile([96, 2, KVH, 3, 96], FP8, name="dgp")
        dge = cpool.tile([96, KVH, 9, 96], FP8, name="dge")
        dwcb = cpool.tile([96, KVH], F32, name="dwcb")
        masks = cpool.tile([128, NK, H], BF16, name="masks")
        ones_r = cpool.tile([1, 128], BF16, name="ones_r")
        ones_c = cpool.tile([128, 1], BF16, name="ones_c")
        sqwarm = cpool.tile([1, 8], F32, name="sqwarm")

        vpad = dpool.tile([BL, N, KVH, 128], BF16, name="vpad")

        # ---- pools ----
        xpool = stack.enter_context(tc.tile_pool(name="xf", bufs=1))
        XF = xpool.tile([128, 2, NK, T], FP8, name="XF")
        wp = stack.enter_context(tc.tile_pool(name="work", bufs=1))
        pmm = stack.enter_context(tc.tile_pool(name="pmm", bufs=1, space="PSUM"))
        pa = stack.enter_context(tc.tile_pool(name="pa", bufs=1, space="PSUM"))

        # ---- prologue machinery: load x, transpose on PE, evacuate as fp8
        # hi/lo into XF. Stages are interleaved with batch-0 K/V tiles so the
        # ACT/DVE evacuation queue never runs ahead of the GEMM consumers.
        prep = stack.enter_context(tc.tile_pool(name="prep", bufs=3))

        def prep_stage(i):
            stage = prep.tile([128, DIM], BF16, name="stage", tag="stage")
            nc.gpsimd.dma_start(out=stage[:],
                                in_=x_in[128 * i:128 * (i + 1), :])
            if i == 0:
                nc.sync.dma_start(out=eye[:], in_=eye_in[:])
                nc.vector.memset(ones_r[:], 1.0)
                nc.vector.memset(ones_c[:], 1.0)
                nc.vector.memset(sqwarm[:], 1.0)
                # warm the Sqrt activation table off the critical path
                nc.scalar.activation(sqwarm[:], sqwarm[:], AF.Sqrt)
            elif i == 1:
                nc.sync.dma_start(out=wkvfk[:], in_=wkvfk_in[:])
                nc.sync.dma_start(out=wkv8rk[:], in_=wkv8rk_in[:])
                nc.sync.dma_start(out=kvb64[:], in_=kvb64_in[:])
            elif i == 3:
                nc.sync.dma_start(out=wkvfv[:], in_=wkvfv_in[:])
                nc.sync.dma_start(out=wkv8rv[:], in_=wkv8rv_in[:])
            elif i == 5:
                for j in range(NK):
                    nc.sync.dma_start(out=wqfj[:, j], in_=wqfj_in[:, j])
                nc.sync.dma_start(out=wq8r[:], in_=wq8r_in[:])
                nc.sync.dma_start(out=wqb[:], in_=wqb_in[:])
            elif i == 9:
                for k in range(NK):
                    nc.sync.dma_start(out=PWT[k][:],
                                      in_=pwT_in[128 * k:128 * (k + 1), :])
                nc.sync.dma_start(out=pjb_bc[:], in_=pjb_bc_in[:])
            elif i == 11:
                nc.sync.dma_start(out=dgp[:], in_=dgp_in[:])
                nc.sync.dma_start(out=dge[:], in_=dge_in[:])
                nc.sync.dma_start(out=dwcb[:], in_=dwcb_in[:])
                nc.sync.dma_start(out=masks[:], in_=masks_in[:])
            pt_a = pa.tile([128, 1024], BF16, name="pt_a", tag="pa", bufs=2)
            for k in range(8):
                nc.tensor.transpose(pt_a[:, 128 * k:128 * (k + 1)],
                                    stage[:, 128 * k:128 * (k + 1)], eye[:])
            pt_b = pa.tile([128, 128], BF16, name="pt_b", tag="pa", bufs=2)
            nc.tensor.transpose(pt_b[:], stage[:, 1024:1152], eye[:])
            cs = slice(128 * i, 128 * (i + 1))
            pt3 = pt_a[:].rearrange("p (k c) -> p k c", c=128)
            # hi casts (ACT), slices 0-7 slot1, slice 8 slot0
            nc.scalar.activation(XF[:, 1, 0:8, cs], pt3, AF.Copy)
            nc.scalar.activation(XF[:, 0, 8, cs], pt_b[:], AF.Copy)
            # lo = psum - hi (DVE), slices 0-7 slot0, slice 8 slot1
            nc.vector.tensor_tensor(out=XF[:, 0, 0:8, cs], in0=pt3,
                                    in1=XF[:, 1, 0:8, cs], op=ALU.subtract)
            nc.vector.tensor_tensor(out=XF[:, 1, 8, cs], in0=pt_b[:],
                                    in1=XF[:, 0, 8, cs], op=ALU.subtract)

        cn = _St()
        cn.wqfj, cn.wq8r, cn.wqb = wqfj, wq8r, wqb
        cn.wkvfk, cn.wkvfv, cn.wkv8rk, cn.wkv8rv = wkvfk, wkvfv, wkv8rk, wkv8rv
        cn.kvb64, cn.PWT, cn.pjb_bc = kvb64, PWT, pjb_bc
        cn.dgp, cn.dge, cn.dwcb, cn.masks = dgp, dge, dwcb, masks
        cn.ones_r, cn.ones_c, cn.XF, cn.vpad, cn.y_out = ones_r, ones_c, XF, vpad, y_out

        p0 = _phases(nc, 0, wp, pmm, pa, cn)
        p1 = _phases(nc, 1, wp, pmm, pa, cn)
        # x stages feed b0's K/V tiles just-in-time, spread across both loops
        prep_stage(0)
        prep_stage(1)
        p0["k_begin"]()
        for t in range(TT):
            prep_stage(t + 2)
            p0["k_tile"](t)
        for t in range(TT):
            if t < 6:
                prep_stage(10 + t)
            p0["v_tile"](t)
        p0["v_end"]()
        p0["vt"](); p0["e1"](); p0["q"](); p0["dwc"]()
        p1["k_begin"]()
        for t in range(TT):
            p1["k_tile"](t)
        p0["norms"](); p0["e2"](0); p0["e2"](1)
        for t in range(TT):
            p1["v_tile"](t)
        p1["v_end"]()
        p1["vt"](); p1["e1"]()
        p0["proj"](0, 6)
        p1["q"](); p1["dwc"]()
        p0["proj"](6, 8)
        p1["norms"](); p1["e2"](0); p1["proj"](0, 4)
        p1["e2"](1); p1["proj"](4, 8)

    nc.compile()
    return nc


def _phases(nc, b, wp, pmm, pa, cn):
    st = _St()
    XF = cn.XF

    def emit_kv_half(t, vhalf, out_pk):
        t0 = b * N + 128 * t
        wf = cn.wkvfv if vhalf else cn.wkvfk
        w8 = cn.wkv8rv if vhalf else cn.wkv8rk
        for ci, cc in ((0, 0), (192, 192)):
            dst = out_pk[:, ci:ci + 192]
            for a in range(4):
                nc.tensor.matmul(dst, XF[:, 1, 2 * a:2 * a + 2, t0:t0 + 128],
                                 wf[:, 0, 2 * a:2 * a + 2, cc:cc + 192],
                                 start=(a == 0), stop=False, perf_mode=DR)
            nc.tensor.matmul(dst, XF[:, :, 8, t0:t0 + 128],
                             wf[:, :, 8, cc:cc + 192],
                             start=False, stop=False, perf_mode=DR)
            for k in range(8):
                nc.tensor.matmul(dst, XF[:, :, k, t0:t0 + 128],
                                 wf[:, :, k, cc:cc + 192],
                                 start=False, stop=False, perf_mode=DR)
            nc.tensor.matmul(dst, XF[:, :, 8, t0:t0 + 128],
                             w8[:, :, cc:cc + 192],
                             start=False, stop=False, perf_mode=DR)
            bc = 384 * vhalf + cc
            nc.tensor.matmul(dst, cn.ones_r[:], cn.kvb64[:, bc:bc + 192],
                             start=False, stop=True)

    def ph_k_begin():
        st.k3 = [wp.tile([128, 384], BF16, name=f"k3_{t}", tag=f"k3_{t}")
                 for t in range(TT)]
        st.vv = [wp.tile([128, 384], BF16, name=f"v_{t}", tag=f"v_{t}")
                 for t in range(TT)]
        st.acc2k = wp.tile([128, KVH * TT], F32, name="acc2k", tag="acc2k", bufs=2)
        st.uk2s = []

    def ph_k_tile(t):
        pk = pmm.tile([128, 512], F32, name="pk", tag="pmm", bufs=3)
        emit_kv_half(t, 0, pk)
        if t == 0:
            st.acc1kr = pmm.tile([1, 384], F32, name="acc1kr", tag="pnorm",
                                 bufs=1)
        if t >= 2:
            # row-accumulate acc1k at a 2-tile lag so PE never waits on ACT
            nc.tensor.matmul(st.acc1kr[:], cn.ones_c[:], st.uk2s[t - 2][:],
                             start=(t == 2), stop=False)
        uk = wp.tile([128, 384], BF16, name="uk", tag="uk", bufs=2)
        nc.scalar.activation(uk[:], pk[:, 0:384], AF.Relu, scale=1.0 / SW)
        uk2 = wp.tile([128, 384], BF16, name="uk2", tag="uk2", bufs=3)
        st.uk2s.append(uk2)
        nc.scalar.activation(uk2[:], uk[:], AF.Square)
        nc.vector.tensor_mul(st.k3[t][:], uk2[:], uk[:])
        uk6 = wp.tile([128, 384], BF16, name="uk6", tag="uk6", bufs=2)
        nc.vector.tensor_mul(uk6[:], st.k3[t][:], st.k3[t][:])
        for g in range(KVH):
            nc.vector.tensor_reduce(st.acc2k[:, g * TT + t:g * TT + t + 1],
                                    uk6[:, 96 * g:96 * (g + 1)],
                                    axis=AX.X, op=ALU.add)

    def ph_v_tile(t):
        pv = pmm.tile([128, 512], F32, name="pv", tag="pmm", bufs=3)
        if t < 2:
            # flush the lagged acc1k row-accumulation
            nc.tensor.matmul(st.acc1kr[:], cn.ones_c[:],
                             st.uk2s[TT - 2 + t][:],
                             start=False, stop=(t == 1))
        emit_kv_half(t, 1, pv)
        nc.scalar.activation(st.vv[t][:], pv[:, 0:384], AF.Copy, scale=1.0 / SW)
        nc.sync.dma_start(
            out=cn.vpad[b, 128 * t:128 * (t + 1), :, 0:96],
            in_=st.vv[t][:].rearrange("p (k d) -> p k d", k=KVH))

    def ph_v_end():
        # k-side acc1 group sums: free the pnorm psum row early
        st.kred1 = wp.tile([1, KVH], F32, name="kred1", tag="kred1", bufs=2)
        nc.vector.tensor_reduce(st.kred1[:],
                                st.acc1kr[:].rearrange("a (k d) -> a k d", k=KVH),
                                axis=AX.X, op=ALU.add)

    def ph_vt():
        st.vT8 = []
        for g in range(KVH):
            vT = wp.tile([128, N], BF16, name="vTd", tag="vTd", bufs=2)
            nc.sync.dma_start(out=vT[:], in_=cn.vpad[b, :, g, :], transpose=True)
            v8 = wp.tile([96, N], FP8, name="v8", tag=f"v8_{g}")
            if g % 2 == 0:
                nc.scalar.activation(v8[:], vT[0:96, :], AF.Copy)
            else:
                nc.vector.tensor_copy(v8[:], vT[0:96, :])
            st.vT8.append(v8)

    def ph_e1():
        st.kvu = [wp.tile([96, 96], BF16, name=f"kvu_{g}", tag=f"kvu_{g}")
                  for g in range(KVH)]
        for g in range(KVH):
            pk_t = pa.tile([96, 96], F32, name="pkvt", tag="pa", bufs=2)
            for t in range(TT):
                nc.tensor.matmul(pk_t[:], st.k3[t][:, 96 * g:96 * (g + 1)],
                                 st.vv[t][:, 96 * g:96 * (g + 1)],
                                 start=(t == 0), stop=(t == TT - 1))
            nc.vector.tensor_copy(st.kvu[g][:], pk_t[:])

    def ph_q():
        st.acc1q = wp.tile([128, NK * CH], F32, name="acc1q", tag="acc1q")
        st.acc2q = wp.tile([128, NK * CH], F32, name="acc2q", tag="acc2q")
        st.q3 = [wp.tile([128, N], BF16, name=f"q3_{j}", tag=f"q3_{j}")
                 for j in range(NK)]
        wqfj, wq8r = cn.wqfj, cn.wq8r
        for c2 in range(CH):
            for j in range(NK):
                pq = pmm.tile([128, 512], F32, name="pq", tag="pmm", bufs=3)
                for sub in range(2):
                    t0 = b * N + 512 * c2 + 256 * sub
                    dst = pq[:, 256 * sub:256 * (sub + 1)]
                    for a in range(4):
                        nc.tensor.matmul(dst, wqfj[:, j, 0, 2 * a:2 * a + 2, :],
                                         XF[:, 1, 2 * a:2 * a + 2, t0:t0 + 256],
                                         start=(a == 0), stop=False, perf_mode=DR)
                    nc.tensor.matmul(dst, wqfj[:, j, :, 8, :],
                                     XF[:, :, 8, t0:t0 + 256],
                                     start=False, stop=False, perf_mode=DR)
                    for k in range(8):
                        nc.tensor.matmul(dst, wqfj[:, j, :, k, :],
                                         XF[:, :, k, t0:t0 + 256],
                                         start=False, stop=False, perf_mode=DR)
                    nc.tensor.matmul(dst, wq8r[:, :, 128 * j:128 * (j + 1)],
                                     XF[:, :, 8, t0:t0 + 256],
                                     start=False, stop=(sub == 1), perf_mode=DR)
                u = wp.tile([128, 512], BF16, name="u", tag="u", bufs=2)
                nc.scalar.activation(u[:], pq[:], AF.Relu, scale=1.0 / SW,
                                     bias=cn.wqb[:, j:j + 1])
                u2 = wp.tile([128, 512], BF16, name="u2", tag="u2", bufs=2)
                col = j * CH + c2
                nc.scalar.activation(u2[:], u[:], AF.Square,
                                     accum_out=st.acc1q[:, col:col + 1])
                q3s = st.q3[j][:, 512 * c2:512 * (c2 + 1)]
                nc.vector.tensor_mul(q3s, u2[:], u[:])
                u6 = wp.tile([128, 512], BF16, name="u6", tag="u6", bufs=2)
                nc.vector.tensor_mul(u6[:], q3s, q3s)
                nc.vector.tensor_reduce(st.acc2q[:, col:col + 1], u6[:],
                                        axis=AX.X, op=ALU.add)

    def ph_dwc():
        st.vdwc = [wp.tile([96, N], BF16, name=f"vdwc_{g}", tag=f"vdwc_{g}")
                   for g in range(KVH)]
        for g in range(KVH):
            v3 = st.vT8[g][:].rearrange("p (y x) -> p y x", y=32)
            for hf in range(2):
                pd = pmm.tile([96, 512], F32, name="pd", tag="pdw", bufs=2)
                p3 = pd[:].rearrange("p (y x) -> p y x", y=16)
                mms = []
                for dxi, dx in enumerate((-1, 0, 1)):
                    x0, x1 = max(0, -dx), 32 - max(0, dx)
                    mms.append((cn.dge[:, g, 3 + dxi, :],
                                v3[0:96, 16 * hf:16 * hf + 16, x0 + dx:x1 + dx],
                                p3[:, 0:16, x0:x1], None))
                ya0 = max(1, 16 * hf)
                ya1 = min(31, 16 * hf + 16)
                for dxi, dx in enumerate((-1, 0, 1)):
                    x0, x1 = max(0, -dx), 32 - max(0, dx)
                    base = v3[0:96, ya0 - 1:ya1 - 1, x0 + dx:x1 + dx]
                    rhs = _ins_dim(base, 64, 2)
                    mms.append((cn.dgp[:, :, g, dxi, :], rhs,
                                p3[:, ya0 - 16 * hf:ya1 - 16 * hf, x0:x1], DR))
                for dxi, dx in enumerate((-1, 0, 1)):
                    x0, x1 = max(0, -dx), 32 - max(0, dx)
                    if hf == 0:  # y=0, tap dy=+1
                        mms.append((cn.dge[:, g, 6 + dxi, :],
                                    v3[0:96, 1:2, x0 + dx:x1 + dx],
                                    p3[:, 0:1, x0:x1], None))
                    else:        # y=31, tap dy=-1
                        mms.append((cn.dge[:, g, dxi, :],
                                    v3[0:96, 30:31, x0 + dx:x1 + dx],
                                    p3[:, 15:16, x0:x1], None))
                for mi, (lhsT, rhs, out, pm) in enumerate(mms):
                    nc.tensor.matmul(out, lhsT, rhs, start=(mi == 0),
                                     stop=(mi == len(mms) - 1), perf_mode=pm)
                nc.scalar.activation(st.vdwc[g][:, 512 * hf:512 * (hf + 1)],
                                     pd[:], AF.Identity, scale=1.0 / SW,
                                     bias=cn.dwcb[:, g:g + 1])

    def ph_norms():
        sq_rows = []
        for ai, acc in enumerate((st.acc1q, st.acc2q)):
            accs = wp.tile([128, NK], F32, name="accs", tag="accs", bufs=2)
            av = acc[:, 0:NK * CH].rearrange("p (j c) -> p j c", c=CH)
            nc.vector.tensor_add(accs[:], av[:, :, 0], av[:, :, 1])
            accsb = wp.tile([128, NK], BF16, name="accsb", tag="accsb", bufs=2)
            nc.vector.tensor_copy(accsb[:], accs[:])
            psn = pa.tile([1, H], F32, name="psn", tag="pa", bufs=2)
            for j in range(NK):
                nc.tensor.matmul(psn[:], accsb[:, j:j + 1], cn.masks[:, j, :],
                                 start=(j == 0), stop=(j == NK - 1))
            srow = wp.tile([1, H], F32, name="srow", tag="srow", bufs=4)
            nc.vector.tensor_copy(srow[:], psn[:])
            sq_rows.append(srow)
        acc2kb = wp.tile([128, KVH * TT], BF16, name="acc2kb", tag="acc2kb",
                         bufs=2)
        nc.vector.tensor_copy(acc2kb[:], st.acc2k[:])
        psk = pa.tile([1, KVH * TT], F32, name="psk", tag="pa", bufs=2)
        nc.tensor.matmul(psk[:], cn.ones_c[:], acc2kb[:], start=True, stop=True)
        krow = wp.tile([1, KVH * TT], F32, name="krow", tag="krow", bufs=2)
        nc.vector.tensor_copy(krow[:], psk[:])
        kred2 = wp.tile([1, KVH], F32, name="kred2", tag="kred2", bufs=2)
        nc.vector.tensor_reduce(kred2[:],
                                krow[:].rearrange("a (k t) -> a k t", k=KVH),
                                axis=AX.X, op=ALU.add)
        sk_rows = [st.kred1, kred2]

        def _f_row(s1, s2, width, tagp):
            se = wp.tile([1, width], F32, name="se", tag=f"se{tagp}", bufs=2)
            nc.vector.tensor_scalar_add(se[:], s2[:], 1e-30)
            rc = wp.tile([1, width], F32, name="rc", tag=f"rc{tagp}", bufs=2)
            nc.vector.reciprocal(rc[:], se[:])
            rt = wp.tile([1, width], F32, name="rt", tag=f"rt{tagp}", bufs=2)
            nc.vector.tensor_mul(rt[:], s1[:], rc[:])
            fr = wp.tile([1, width], F32, name="fr", tag=f"fr{tagp}", bufs=2)
            nc.scalar.activation(fr[:], rt[:], AF.Sqrt)
            return fr

        fq = _f_row(sq_rows[0], sq_rows[1], H, "q")
        fk = _f_row(sk_rows[0], sk_rows[1], KVH, "k")
        fk12 = wp.tile([1, H], F32, name="fk12", tag="fk12", bufs=2)
        for g in range(3):
            nc.vector.tensor_copy(fk12[:, 4 * g:4 * (g + 1)], fk[:])
        grow = wp.tile([1, H], F32, name="grow", tag="grow", bufs=2)
        nc.vector.tensor_mul(grow[:], fq[:], fk12[:])
        gb = wp.tile([96, H], F32, name="gb", tag="gb", bufs=2)
        nc.gpsimd.partition_broadcast(gb[:], grow[:], channels=96)
        st.kvp = [wp.tile([96, 96], BF16, name=f"kvp_{h}", tag=f"kvp_{h}")
                  for h in range(H)]
        for h in range(H):
            nc.vector.tensor_scalar_mul(st.kvp[h][:], st.kvu[h % KVH][:],
                                        gb[:, h:h + 1])

    def ph_e2(c2):
        if c2 == 0:
            st.OT = [wp.tile([128, N], BF16, name=f"OT_{j}", tag=f"OT_{j}")
                     for j in range(NK)]
        for h in range(H):
            pieces = _head_pieces(h)
            if len(pieces) == 1:
                j0, r00, _, _ = pieces[0]
                rhs = st.q3[j0][r00:r00 + 96, 512 * c2:512 * (c2 + 1)]
            else:
                qh = wp.tile([96, 512], BF16, name="qh", tag="qh", bufs=4)
                for pi, (j, r0, rr, cnt) in enumerate(pieces):
                    src_ap = st.q3[j][r0:r0 + cnt, 512 * c2:512 * (c2 + 1)]
                    if (h + pi) % 2 == 0:
                        nc.vector.tensor_copy(qh[rr:rr + cnt, :], src_ap)
                    else:
                        nc.scalar.copy(qh[rr:rr + cnt, :], src_ap)
                rhs = qh[:]
            pa_t = pa.tile([96, 512], F32, name="pat", tag="pa", bufs=2)
            nc.tensor.matmul(pa_t[:], st.kvp[h][:], rhs, start=True,
                             stop=True)
            if len(pieces) == 1:
                j0, r00, _, _ = pieces[0]
                nc.vector.tensor_tensor(
                    out=st.OT[j0][r00:r00 + 96, 512 * c2:512 * (c2 + 1)],
                    in0=pa_t[:],
                    in1=st.vdwc[h % KVH][:, 512 * c2:512 * (c2 + 1)],
                    op=ALU.add)
            else:
                pac = wp.tile([96, 512], BF16, name="pac", tag="pac", bufs=4)
                nc.scalar.copy(pac[:], pa_t[:])
                for (j, r0, rr, cnt) in pieces:
                    nc.vector.tensor_tensor(
                        out=st.OT[j][r0:r0 + cnt, 512 * c2:512 * (c2 + 1)],
                        in0=pac[rr:rr + cnt, :],
                        in1=st.vdwc[h % KVH][rr:rr + cnt,
                                             512 * c2:512 * (c2 + 1)],
                        op=ALU.add)

    def ph_proj(ta, tb):
        for t in range(ta, tb):
            for oc in range(3):
                py = pmm.tile([128, 384], F32, name="py", tag="pdw", bufs=2)
                for j in range(NK):
                    nc.tensor.matmul(py[:], st.OT[j][:, 128 * t:128 * (t + 1)],
                                     cn.PWT[j][:, 384 * oc:384 * (oc + 1)],
                                     start=(j == 0), stop=False)
                nc.tensor.matmul(py[:], cn.ones_r[:],
                                 cn.pjb_bc[:, 384 * oc:384 * (oc + 1)],
                                 start=False, stop=True)
                ysb = wp.tile([128, 384], F32, name="ysb", tag="ysb", bufs=3)
                if (t + oc) % 2 == 0:
                    nc.scalar.copy(ysb[:], py[:])
                else:
                    nc.vector.tensor_copy(ysb[:], py[:])
                t0 = b * N + 128 * t
                nc.sync.dma_start(out=cn.y_out[t0:t0 + 128,
                                               384 * oc:384 * (oc + 1)],
                                  in_=ysb[:])

    return dict(k_begin=ph_k_begin, k_tile=ph_k_tile, v_tile=ph_v_tile,
                v_end=ph_v_end, vt=ph_vt, e1=ph_e1, q=ph_q,
                dwc=ph_dwc, norms=ph_norms, e2=ph_e2, proj=ph_proj)


_NC_CACHE = None


def _get_nc():
    global _NC_CACHE
    if _NC_CACHE is None:
        _NC_CACHE = _build_kernel()
    return _NC_CACHE


def _hi_lo(a):
    hi = a.astype(_F8)
    lo = (a - hi.astype(np.float32)).astype(_F8)
    return hi, lo


def _host_consts(wq_w, wq_b, wkv_w, wkv_b, dwc_w, dwc_b, proj_w, proj_b):
    wqT = np.ascontiguousarray(np.asarray(wq_w, np.float32).T) * SW      # [in, out]
    wkvT = np.ascontiguousarray(np.asarray(wkv_w, np.float32).T) * SW    # [in, 768]
    qhi, qlo = _hi_lo(wqT)
    khi, klo = _hi_lo(wkvT)

    # wqfj: [128, j, slot(hi,lo), k, 128]
    wqfj = np.zeros((128, NK, 2, NK, 128), _F8)
    for k in range(NK):
        for j in range(NK):
            wqfj[:, j, 0, k, :] = qhi[128 * k:128 * (k + 1), 128 * j:128 * (j + 1)]
            wqfj[:, j, 1, k, :] = qlo[128 * k:128 * (k + 1), 128 * j:128 * (j + 1)]
    wq8r = np.zeros((128, 2, DIM), _F8)
    wq8r[:, 0, :] = qlo[128 * 8:, :]
    wq8r[:, 1, :] = qhi[128 * 8:, :]

    wkvf = np.zeros((128, 2, NK, 768), _F8)
    for k in range(NK):
        wkvf[:, 0, k, :] = khi[128 * k:128 * (k + 1), :]
        wkvf[:, 1, k, :] = klo[128 * k:128 * (k + 1), :]
    wkv8r = np.zeros((128, 2, 768), _F8)
    wkv8r[:, 0, :] = klo[128 * 8:, :]
    wkv8r[:, 1, :] = khi[128 * 8:, :]
    wkvfk = np.ascontiguousarray(wkvf[:, :, :, 0:384])
    wkvfv = np.ascontiguousarray(wkvf[:, :, :, 384:768])
    wkv8rk = np.ascontiguousarray(wkv8r[:, :, 0:384])
    wkv8rv = np.ascontiguousarray(wkv8r[:, :, 384:768])

    pwT = np.ascontiguousarray(np.asarray(proj_w, np.float32).T).astype(_BF)
    wqb = np.ascontiguousarray(np.asarray(wq_b, np.float32).reshape(NK, 128).T)
    kvb64 = (np.asarray(wkv_b, np.float32).reshape(1, 768) * SW).astype(_BF)
    pjb_bc = np.asarray(proj_b, np.float32).reshape(1, DIM).astype(_BF)

    dw = np.asarray(dwc_w, np.float32).reshape(KVH, 96, 9) * SW  # [g, d, tap]
    dgp = np.zeros((96, 2, KVH, 3, 96), np.float32)
    dge = np.zeros((96, KVH, 9, 96), np.float32)
    for d in range(96):
        for dxi in range(3):
            dgp[d, 0, :, dxi, d] = dw[:, d, 0 + dxi]       # dy=-1 taps 0,1,2
            dgp[d, 1, :, dxi, d] = dw[:, d, 6 + dxi]       # dy=+1 taps 6,7,8
        for ti in range(9):
            dge[d, :, ti, d] = dw[:, d, ti]
    dgp = dgp.astype(_F8)
    dge = dge.astype(_F8)
    dwcb = np.ascontiguousarray(np.asarray(dwc_b, np.float32).reshape(KVH, 96).T)

    mk = np.zeros((128, NK, H), np.float32)
    for j in range(NK):
        for p in range(128):
            f = 128 * j + p
            mk[p, j, f // 96] = 1.0
    masks = mk.astype(_BF)
    eye = np.eye(128, dtype=np.float32).astype(_BF)
    return dict(wqfj=wqfj, wq8r=wq8r, wkvfk=wkvfk, wkvfv=wkvfv, wkv8rk=wkv8rk,
                wkv8rv=wkv8rv, pwT=pwT, wqb=wqb, kvb64=kvb64, pjb_bc=pjb_bc,
                dgp=dgp, dge=dge, dwcb=dwcb, masks=masks, eye=eye)


def kernel(x, wq_w, wq_b, wkv_w, wkv_b, dwc_w, dwc_b, proj_w, proj_b,
           _want_results=False, **_unused):
    nc = _get_nc()
    consts = _host_consts(wq_w, wq_b, wkv_w, wkv_b, dwc_w, dwc_b, proj_w, proj_b)
    x = np.asarray(x, np.float32)
    in_maps = []
    for c in range(NCORES):
        m = dict(consts)
        m["x"] = np.ascontiguousarray(x[BL * c:BL * (c + 1)].reshape(T, DIM))
        in_maps.append(m)
    res = bass_utils.run_bass_kernel_spmd(nc, in_maps, core_ids=list(range(NCORES)))
    y = np.stack([res.results[c]["y"].reshape(BL, N, DIM) for c in range(NCORES)])
    y = y.reshape(B, N, DIM)
    if _want_results:
        return y, res
    return y


# revision 39
# speedup vs baseline: 1.1407x; 1.0494x over previous
"""Trainium2 Bass kernel for DiT focused-linear-attention block (nn_DiT_9259949490457).

Data-parallel over batch: 16 batches -> 8 NeuronCores, 2 batches/core, no collectives.

q and kv GEMMs run in fp8-e4m3 hi/lo split-precision using DoubleRow perf mode
(2 slice-products per 0.5-cycle/row instruction -> 14 DR units vs 18 bf16 units per
K=1152 contraction, with better-than-bf16 accuracy). The hi/lo fp8 cast of x is fused
into the transpose-psum evacuation (ACT writes hi, DVE subtract writes lo). The
depthwise 3x3 conv branch pairs (dy=-1,dy=+1) taps into DoubleRow fp8 matmuls via
custom overlapping access patterns.

The two local batches are phase-interleaved so PE always has matmul work while the
other batch's norm chain / einsum2-evacuation drains on ACT/DVE:
  b0:[K,V,vT,e1,Q,u6,dwc] b1:K b0:[norms,e2] b1:[V,vT,e1] b0:proj[0:6]
  b1:[Q,u6,dwc] b0:proj[6:8] b1:[norms,e2,proj]

Self-contained: hardcodes all shapes; host numpy pre-packs fp8 hi/lo weights
(scaled by 64; 1/64 folded into psum-evacuation activations).
"""

import numpy as np
import ml_dtypes
import bass_rust

import concourse.bacc as bacc
import concourse.mybir as mybir
import concourse.tile as tile
from concourse import bass_utils

F32 = mybir.dt.float32
BF16 = mybir.dt.bfloat16
FP8 = mybir.dt.float8e4
ALU = mybir.AluOpType
AF = mybir.ActivationFunctionType
AX = mybir.AxisListType
DR = mybir.MatmulPerfMode.DoubleRow

NCORES = 8
B, N, DIM = 16, 1024, 1152
H, KVH, HD = 12, 4, 96
BL = B // NCORES          # 2 local batches
T = BL * N                # 2048 local tokens
NK = DIM // 128           # 9 feature K-slices
TT = N // 128             # 8 token tiles per batch
CH = N // 512             # 2 free-dim chunks of 512 per batch
SW = 64.0                 # fp8 weight pre-scale (power of two)

_BF = ml_dtypes.bfloat16
_F8 = ml_dtypes.float8_e4m3fn


def _spanp(b):
    if b % 128 == 0:
        return 128
    if b % 64 == 0:
        return 64
    return 32


def _head_pieces(h):
    out = []
    rr = 0
    while rr < 96:
        gr = 96 * h + rr
        j, r0 = divmod(gr, 128)
        cnt = min(96 - rr, 128 - r0, _spanp(r0), _spanp(rr))
        out.append((j, r0, rr, cnt))
        rr += cnt
    return out


def _ins_dim(ap, stride, count):
    """Insert a free dim [stride, count] right after the partition dim."""
    dims = [list(d) for d in ap.ap]
    new = [dims[0], [stride, count]] + dims[1:]
    return bass_rust.AP(ap.tensor, ap.offset, new)


class _St:
    pass


def _build_kernel():
    nc = bacc.Bacc("TRN2", target_bir_lowering=False, debug=False,
                   enable_asserts=True, num_devices=NCORES)
    x_in = nc.dram_tensor("x", [T, DIM], F32, kind="ExternalInput").ap()
    wqfj_in = nc.dram_tensor("wqfj", [128, NK, 2, NK, 128], FP8, kind="ExternalInput").ap()
    wq8r_in = nc.dram_tensor("wq8r", [128, 2, DIM], FP8, kind="ExternalInput").ap()
    wkvfk_in = nc.dram_tensor("wkvfk", [128, 2, NK, 384], FP8, kind="ExternalInput").ap()
    wkvfv_in = nc.dram_tensor("wkvfv", [128, 2, NK, 384], FP8, kind="ExternalInput").ap()
    wkv8rk_in = nc.dram_tensor("wkv8rk", [128, 2, 384], FP8, kind="ExternalInput").ap()
    wkv8rv_in = nc.dram_tensor("wkv8rv", [128, 2, 384], FP8, kind="ExternalInput").ap()
    pwT_in = nc.dram_tensor("pwT", [DIM, DIM], BF16, kind="ExternalInput").ap()
    wqb_in = nc.dram_tensor("wqb", [128, NK], F32, kind="ExternalInput").ap()
    kvb64_in = nc.dram_tensor("kvb64", [1, 768], BF16, kind="ExternalInput").ap()
    pjb_bc_in = nc.dram_tensor("pjb_bc", [128, DIM], BF16, kind="ExternalInput").ap()
    dgp_in = nc.dram_tensor("dgp", [96, 2, KVH, 3, 96], FP8, kind="ExternalInput").ap()
    dge_in = nc.dram_tensor("dge", [96, KVH, 9, 96], FP8, kind="ExternalInput").ap()
    dwcb_in = nc.dram_tensor("dwcb", [96, KVH], F32, kind="ExternalInput").ap()
    masks_in = nc.dram_tensor("masks", [128, NK, H], BF16, kind="ExternalInput").ap()
    eye_in = nc.dram_tensor("eye", [128, 128], BF16, kind="ExternalInput").ap()
    y_out = nc.dram_tensor("y", [T, DIM], F32, kind="ExternalOutput").ap()

    from contextlib import ExitStack
    with tile.TileContext(nc) as tc, ExitStack() as stack:
        cpool = stack.enter_context(tc.tile_pool(name="const", bufs=1))
        dpool = stack.enter_context(tc.tile_pool(name="dram", bufs=1, space="DRAM"))

        # ---- consts / weights ----
        eye = cpool.tile([128, 128], BF16, name="eye")
        wkvfk = cpool.tile([128, 2, NK, 384], FP8, name="wkvfk")
        wkvfv = cpool.tile([128, 2, NK, 384], FP8, name="wkvfv")
        wkv8rk = cpool.tile([128, 2, 384], FP8, name="wkv8rk")
        wkv8rv = cpool.tile([128, 2, 384], FP8, name="wkv8rv")
        kvb64 = cpool.tile([1, 768], BF16, name="kvb64")
        wqfj = cpool.tile([128, NK, 2, NK, 128], FP8, name="wqfj")
        wq8r = cpool.tile([128, 2, DIM], FP8, name="wq8r")
        wqb = cpool.tile([128, NK], F32, name="wqb")
        PWT = [cpool.tile([128, DIM], BF16, name=f"PWT{k}") for k in range(NK)]
        pjb_bc = cpool.tile([128, DIM], BF16, name="pjb_bc")
        dgp = cpool.tile([96, 2, KVH, 3, 96], FP8, name="dgp")
        dge = cpool.tile([96, KVH, 9, 96], FP8, name="dge")
        dwcb = cpool.tile([96, KVH], F32, name="dwcb")
        masks = cpool.tile([128, NK, H], BF16, name="masks")
        ones_r = cpool.tile([1, 128], BF16, name="ones_r")
        ones_c = cpool.tile([128, 1], BF16, name="ones_c")
        sqwarm = cpool.tile([1, 8], F32, name="sqwarm")

        vpad = dpool.tile([BL, N, KVH, 128], BF16, name="vpad")

        # ---- pools ----
        xpool = stack.enter_context(tc.tile_pool(name="xf", bufs=1))
        XF = xpool.tile([128, 2, NK, T], FP8, name="XF")
        wp = stack.enter_context(tc.tile_pool(name="work", bufs=1))
        pmm = stack.enter_context(tc.tile_pool(name="pmm", bufs=1, space="PSUM"))
        pa = stack.enter_context(tc.tile_pool(name="pa", bufs=1, space="PSUM"))

        # ---- prologue machinery: load x, transpose on PE, evacuate as fp8
        # hi/lo into XF. Stages are interleaved with batch-0 K/V tiles so the
        # ACT/DVE evacuation queue never runs ahead of the GEMM consumers.
        prep = stack.enter_context(tc.tile_pool(name="prep", bufs=3))

        def prep_stage(i):
            stage = prep.tile([128, DIM], BF16, name="stage", tag="stage")
            nc.gpsimd.dma_start(out=stage[:],
                                in_=x_in[128 * i:128 * (i + 1), :])
            if i == 0:
                nc.sync.dma_start(out=eye[:], in_=eye_in[:])
                nc.vector.memset(ones_r[:], 1.0)
                nc.vector.memset(ones_c[:], 1.0)
                nc.vector.memset(sqwarm[:], 1.0)
                # warm the Sqrt activation table off the critical path
                nc.scalar.activation(sqwarm[:], sqwarm[:], AF.Sqrt)
            elif i == 1:
                nc.sync.dma_start(out=wkvfk[:], in_=wkvfk_in[:])
                nc.sync.dma_start(out=wkv8rk[:], in_=wkv8rk_in[:])
                nc.sync.dma_start(out=kvb64[:], in_=kvb64_in[:])
            elif i == 3:
                nc.sync.dma_start(out=wkvfv[:], in_=wkvfv_in[:])
                nc.sync.dma_start(out=wkv8rv[:], in_=wkv8rv_in[:])
            elif i == 5:
                for j in range(NK):
                    nc.sync.dma_start(out=wqfj[:, j], in_=wqfj_in[:, j])
                nc.sync.dma_start(out=wq8r[:], in_=wq8r_in[:])
                nc.sync.dma_start(out=wqb[:], in_=wqb_in[:])
            elif i == 9:
                for k in range(NK):
                    nc.sync.dma_start(out=PWT[k][:],
                                      in_=pwT_in[128 * k:128 * (k + 1), :])
                nc.sync.dma_start(out=pjb_bc[:], in_=pjb_bc_in[:])
            elif i == 11:
                nc.sync.dma_start(out=dgp[:], in_=dgp_in[:])
                nc.sync.dma_start(out=dge[:], in_=dge_in[:])
                nc.sync.dma_start(out=dwcb[:], in_=dwcb_in[:])
                nc.sync.dma_start(out=masks[:], in_=masks_in[:])
            pt_a = pa.tile([128, 1024], BF16, name="pt_a", tag="pa", bufs=2)
            for k in range(8):
                nc.tensor.transpose(pt_a[:, 128 * k:128 * (k + 1)],
                                    stage[:, 128 * k:128 * (k + 1)], eye[:])
            pt_b = pa.tile([128, 128], BF16, name="pt_b", tag="pa", bufs=2)
            nc.tensor.transpose(pt_b[:], stage[:, 1024:1152], eye[:])
            cs = slice(128 * i, 128 * (i + 1))
            pt3 = pt_a[:].rearrange("p (k c) -> p k c", c=128)
            # hi casts (ACT), slices 0-7 slot1, slice 8 slot0
            nc.scalar.activation(XF[:, 1, 0:8, cs], pt3, AF.Copy)
            nc.scalar.activation(XF[:, 0, 8, cs], pt_b[:], AF.Copy)
            # lo = psum - hi (DVE), slices 0-7 slot0, slice 8 slot1
            nc.vector.tensor_tensor(out=XF[:, 0, 0:8, cs], in0=pt3,
                                    in1=XF[:, 1, 0:8, cs], op=ALU.subtract)
            nc.vector.tensor_tensor(out=XF[:, 1, 8, cs], in0=pt_b[:],
                                    in1=XF[:, 0, 8, cs], op=ALU.subtract)

        cn = _St()
        cn.wqfj, cn.wq8r, cn.wqb = wqfj, wq8r, wqb
        cn.wkvfk, cn.wkvfv, cn.wkv8rk, cn.wkv8rv = wkvfk, wkvfv, wkv8rk, wkv8rv
        cn.kvb64, cn.PWT, cn.pjb_bc = kvb64, PWT, pjb_bc
        cn.dgp, cn.dge, cn.dwcb, cn.masks = dgp, dge, dwcb, masks
        cn.ones_r, cn.ones_c, cn.XF, cn.vpad, cn.y_out = ones_r, ones_c, XF, vpad, y_out

        p0 = _phases(nc, 0, wp, pmm, pa, cn)
        p1 = _phases(nc, 1, wp, pmm, pa, cn)
        # x stages feed b0's K/V tiles just-in-time, spread across both loops
        prep_stage(0)
        prep_stage(1)
        p0["k_begin"]()
        for t in range(TT):
            prep_stage(t + 2)
            p0["k_tile"](t)
        for t in range(TT):
            if t < 6:
                prep_stage(10 + t)
            p0["v_tile"](t)
        p0["v_end"]()
        p0["vt"](); p0["e1"](); p0["q"](); p0["dwc"]()
        p1["k_begin"]()
        for t in range(TT):
            p1["k_tile"](t)
        p0["norms"](); p0["e2"](0); p0["e2"](1)
        for t in range(TT):
            p1["v_tile"](t)
        p1["v_end"]()
        p1["vt"](); p1["e1"]()
        p0["proj"](0, 6)
        p1["q"](); p1["dwc"]()
        p0["proj"](6, 8)
        p1["norms"](); p1["e2"](0); p1["proj"](0, 4)
        p1["e2"](1); p1["proj"](4, 8)

    nc.compile()
    return nc


def _phases(nc, b, wp, pmm, pa, cn):
    st = _St()
    XF = cn.XF

    def emit_kv_half(t, vhalf, out_pk):
        t0 = b * N + 128 * t
        wf = cn.wkvfv if vhalf else cn.wkvfk
        w8 = cn.wkv8rv if vhalf else cn.wkv8rk
        for ci, cc in ((0, 0), (192, 192)):
            dst = out_pk[:, ci:ci + 192]
            for a in range(4):
                nc.tensor.matmul(dst, XF[:, 1, 2 * a:2 * a + 2, t0:t0 + 128],
                                 wf[:, 0, 2 * a:2 * a + 2, cc:cc + 192],
                                 start=(a == 0), stop=False, perf_mode=DR)
            nc.tensor.matmul(dst, XF[:, :, 8, t0:t0 + 128],
                             wf[:, :, 8, cc:cc + 192],
                             start=False, stop=False, perf_mode=DR)
            for k in range(8):
                nc.tensor.matmul(dst, XF[:, :, k, t0:t0 + 128],
                                 wf[:, :, k, cc:cc + 192],
                                 start=False, stop=False, perf_mode=DR)
            nc.tensor.matmul(dst, XF[:, :, 8, t0:t0 + 128],
                             w8[:, :, cc:cc + 192],
                             start=False, stop=False, perf_mode=DR)
            bc = 384 * vhalf + cc
            nc.tensor.matmul(dst, cn.ones_r[:], cn.kvb64[:, bc:bc + 192],
                             start=False, stop=True)

    def ph_k_begin():
        st.k3 = [wp.tile([128, 384], BF16, name=f"k3_{t}", tag=f"k3_{t}")
                 for t in range(TT)]
        st.vv = [wp.tile([128, 384], BF16, name=f"v_{t}", tag=f"v_{t}")
                 for t in range(TT)]
        st.acc2k = wp.tile([128, KVH * TT], F32, name="acc2k", tag="acc2k", bufs=2)
        st.uk2s = []

    def ph_k_tile(t):
        pk = pmm.tile([128, 512], F32, name="pk", tag="pmm", bufs=3)
        emit_kv_half(t, 0, pk)
        if t == 0:
            st.acc1kr = pmm.tile([1, 384], F32, name="acc1kr", tag="pnorm",
                                 bufs=1)
        if t >= 2:
            # row-accumulate acc1k at a 2-tile lag so PE never waits on ACT
            nc.tensor.matmul(st.acc1kr[:], cn.ones_c[:], st.uk2s[t - 2][:],
                             start=(t == 2), stop=False)
        uk = wp.tile([128, 384], BF16, name="uk", tag="uk", bufs=2)
        nc.scalar.activation(uk[:], pk[:, 0:384], AF.Relu, scale=1.0 / SW)
        uk2 = wp.tile([128, 384], BF16, name="uk2", tag="uk2", bufs=3)
        st.uk2s.append(uk2)
        nc.scalar.activation(uk2[:], uk[:], AF.Square)
        nc.vector.tensor_mul(st.k3[t][:], uk2[:], uk[:])
        uk6 = wp.tile([128, 384], BF16, name="uk6", tag="uk6", bufs=2)
        nc.vector.tensor_mul(uk6[:], st.k3[t][:], st.k3[t][:])
        for g in range(KVH):
            nc.vector.tensor_reduce(st.acc2k[:, g * TT + t:g * TT + t + 1],
                                    uk6[:, 96 * g:96 * (g + 1)],
                                    axis=AX.X, op=ALU.add)

    def ph_v_tile(t):
        pv = pmm.tile([128, 512], F32, name="pv", tag="pmm", bufs=3)
        if t < 2:
            # flush the lagged acc1k row-accumulation
            nc.tensor.matmul(st.acc1kr[:], cn.ones_c[:],
                             st.uk2s[TT - 2 + t][:],
                             start=False, stop=(t == 1))
        emit_kv_half(t, 1, pv)
        nc.scalar.activation(st.vv[t][:], pv[:, 0:384], AF.Copy, scale=1.0 / SW)
        nc.sync.dma_start(
            out=cn.vpad[b, 128 * t:128 * (t + 1), :, 0:96],
            in_=st.vv[t][:].rearrange("p (k d) -> p k d", k=KVH))

    def ph_v_end():
        # k-side acc1 group sums: free the pnorm psum row early
        st.kred1 = wp.tile([1, KVH], F32, name="kred1", tag="kred1", bufs=2)
        nc.vector.tensor_reduce(st.kred1[:],
                                st.acc1kr[:].rearrange("a (k d) -> a k d", k=KVH),
                                axis=AX.X, op=ALU.add)

    def ph_vt():
        st.vT8 = []
        for g in range(KVH):
            vT = wp.tile([128, N], BF16, name="vTd", tag="vTd", bufs=2)
            nc.sync.dma_start(out=vT[:], in_=cn.vpad[b, :, g, :], transpose=True)
            v8 = wp.tile([96, N], FP8, name="v8", tag=f"v8_{g}")
            if g % 2 == 0:
                nc.scalar.activation(v8[:], vT[0:96, :], AF.Copy)
            else:
                nc.vector.tensor_copy(v8[:], vT[0:96, :])
            st.vT8.append(v8)

    def ph_e1():
        st.kvu = [wp.tile([96, 96], BF16, name=f"kvu_{g}", tag=f"kvu_{g}")
                  for g in range(KVH)]
        for g in range(KVH):
            pk_t = pa.tile([96, 96], F32, name="pkvt", tag="pa", bufs=2)
            for t in range(TT):
                nc.tensor.matmul(pk_t[:], st.k3[t][:, 96 * g:96 * (g + 1)],
                                 st.vv[t][:, 96 * g:96 * (g + 1)],
                                 start=(t == 0), stop=(t == TT - 1))
            nc.vector.tensor_copy(st.kvu[g][:], pk_t[:])

    def ph_q():
        st.acc1q = wp.tile([128, NK * CH], F32, name="acc1q", tag="acc1q")
        st.acc2q = wp.tile([128, NK * CH], F32, name="acc2q", tag="acc2q")
        st.q3 = [wp.tile([128, N], BF16, name=f"q3_{j}", tag=f"q3_{j}")
                 for j in range(NK)]
        wqfj, wq8r = cn.wqfj, cn.wq8r
        for c2 in range(CH):
            for j in range(NK):
                pq = pmm.tile([128, 512], F32, name="pq", tag="pmm", bufs=3)
                for sub in range(2):
                    t0 = b * N + 512 * c2 + 256 * sub
                    dst = pq[:, 256 * sub:256 * (sub + 1)]
                    for a in range(4):
                        nc.tensor.matmul(dst, wqfj[:, j, 0, 2 * a:2 * a + 2, :],
                                         XF[:, 1, 2 * a:2 * a + 2, t0:t0 + 256],
                                         start=(a == 0), stop=False, perf_mode=DR)
                    nc.tensor.matmul(dst, wqfj[:, j, :, 8, :],
                                     XF[:, :, 8, t0:t0 + 256],
                                     start=False, stop=False, perf_mode=DR)
                    for k in range(8):
                        nc.tensor.matmul(dst, wqfj[:, j, :, k, :],
                                         XF[:, :, k, t0:t0 + 256],
                                         start=False, stop=False, perf_mode=DR)
                    nc.tensor.matmul(dst, wq8r[:, :, 128 * j:128 * (j + 1)],
                                     XF[:, :, 8, t0:t0 + 256],
                                     start=False, stop=(sub == 1), perf_mode=DR)
                u = wp.tile([128, 512], BF16, name="u", tag="u", bufs=2)
                nc.scalar.activation(u[:], pq[:], AF.Relu, scale=1.0 / SW,
                                     bias=cn.wqb[:, j:j + 1])
                u2 = wp.tile([128, 512], BF16, name="u2", tag="u2", bufs=2)
                col = j * CH + c2
                nc.scalar.activation(u2[:], u[:], AF.Square,
                                     accum_out=st.acc1q[:, col:col + 1])
                q3s = st.q3[j][:, 512 * c2:512 * (c2 + 1)]
                nc.vector.tensor_mul(q3s, u2[:], u[:])
                u6 = wp.tile([128, 512], BF16, name="u6", tag="u6", bufs=2)
                nc.vector.tensor_mul(u6[:], q3s, q3s)
                nc.vector.tensor_reduce(st.acc2q[:, col:col + 1], u6[:],
                                        axis=AX.X, op=ALU.add)

    def ph_dwc():
        st.vdwc = [wp.tile([96, N], BF16, name=f"vdwc_{g}", tag=f"vdwc_{g}")
                   for g in range(KVH)]
        for g in range(KVH):
            v3 = st.vT8[g][:].rearrange("p (y x) -> p y x", y=32)
            for hf in range(2):
                pd = pmm.tile([96, 512], F32, name="pd", tag="pdw", bufs=2)
                p3 = pd[:].rearrange("p (y x) -> p y x", y=16)
                mms = []
                for dxi, dx in enumerate((-1, 0, 1)):
                    x0, x1 = max(0, -dx), 32 - max(0, dx)
                    mms.append((cn.dge[:, g, 3 + dxi, :],
                                v3[0:96, 16 * hf:16 * hf + 16, x0 + dx:x1 + dx],
                                p3[:, 0:16, x0:x1], None))
                ya0 = max(1, 16 * hf)
                ya1 = min(31, 16 * hf + 16)
                for dxi, dx in enumerate((-1, 0, 1)):
                    x0, x1 = max(0, -dx), 32 - max(0, dx)
                    base = v3[0:96, ya0 - 1:ya1 - 1, x0 + dx:x1 + dx]
                    rhs = _ins_dim(base, 64, 2)
                    mms.append((cn.dgp[:, :, g, dxi, :], rhs,
                                p3[:, ya0 - 16 * hf:ya1 - 16 * hf, x0:x1], DR))
                for dxi, dx in enumerate((-1, 0, 1)):
                    x0, x1 = max(0, -dx), 32 - max(0, dx)
                    if hf == 0:  # y=0, tap dy=+1
                        mms.append((cn.dge[:, g, 6 + dxi, :],
                                    v3[0:96, 1:2, x0 + dx:x1 + dx],
                                    p3[:, 0:1, x0:x1], None))
                    else:        # y=31, tap dy=-1
                        mms.append((cn.dge[:, g, dxi, :],
                                    v3[0:96, 30:31, x0 + dx:x1 + dx],
                                    p3[:, 15:16, x0:x1], None))
                for mi, (lhsT, rhs, out, pm) in enumerate(mms):
                    nc.tensor.matmul(out, lhsT, rhs, start=(mi == 0),
                                     stop=(mi == len(mms) - 1), perf_mode=pm)
                nc.scalar.activation(st.vdwc[g][:, 512 * hf:512 * (hf + 1)],
                                     pd[:], AF.Identity, scale=1.0 / SW,
                                     bias=cn.dwcb[:, g:g + 1])

    def ph_norms():
        sq_rows = []
        for ai, acc in enumerate((st.acc1q, st.acc2q)):
            accs = wp.tile([128, NK], F32, name="accs", tag="accs", bufs=2)
            av = acc[:, 0:NK * CH].rearrange("p (j c) -> p j c", c=CH)
            nc.vector.tensor_add(accs[:], av[:, :, 0], av[:, :, 1])
            accsb = wp.tile([128, NK], BF16, name="accsb", tag="accsb", bufs=2)
            nc.vector.tensor_copy(accsb[:], accs[:])
            psn = pa.tile([1, H], F32, name="psn", tag="pa", bufs=2)
            for j in range(NK):
                nc.tensor.matmul(psn[:], accsb[:, j:j + 1], cn.masks[:, j, :],
                                 start=(j == 0), stop=(j == NK - 1))
            srow = wp.tile([1, H], F32, name="srow", tag="srow", bufs=4)
            nc.vector.tensor_copy(srow[:], psn[:])
            sq_rows.append(srow)
        acc2kb = wp.tile([128, KVH * TT], BF16, name="acc2kb", tag="acc2kb",
                         bufs=2)
        nc.vector.tensor_copy(acc2kb[:], st.acc2k[:])
        psk = pa.tile([1, KVH * TT], F32, name="psk", tag="pa", bufs=2)
        nc.tensor.matmul(psk[:], cn.ones_c[:], acc2kb[:], start=True, stop=True)
        krow = wp.tile([1, KVH * TT], F32, name="krow", tag="krow", bufs=2)
        nc.vector.tensor_copy(krow[:], psk[:])
        kred2 = wp.tile([1, KVH], F32, name="kred2", tag="kred2", bufs=2)
        nc.vector.tensor_reduce(kred2[:],
                                krow[:].rearrange("a (k t) -> a k t", k=KVH),
                                axis=AX.X, op=ALU.add)
        sk_rows = [st.kred1, kred2]

        def _f_row(s1, s2, width, tagp):
            se = wp.tile([1, width], F32, name="se", tag=f"se{tagp}", bufs=2)
            nc.vector.tensor_scalar_add(se[:], s2[:], 1e-30)
            rc = wp.tile([1, width], F32, name="rc", tag=f"rc{tagp}", bufs=2)
            nc.vector.reciprocal(rc[:], se[:])
            rt = wp.tile([1, width], F32, name="rt", tag=f"rt{tagp}", bufs=2)
            nc.vector.tensor_mul(rt[:], s1[:], rc[:])
            fr = wp.tile([1, width], F32, name="fr", tag=f"fr{tagp}", bufs=2)
            nc.scalar.activation(fr[:], rt[:], AF.Sqrt)
            return fr

        fq = _f_row(sq_rows[0], sq_rows[1], H, "q")
        fk = _f_row(sk_rows[0], sk_rows[1], KVH, "k")
        fk12 = wp.tile([1, H], F32, name="fk12", tag="fk12", bufs=2)
        for g in range(3):
            nc.vector.tensor_copy(fk12[:, 4 * g:4 * (g + 1)], fk[:])
        grow = wp.tile([1, H], F32, name="grow", tag="grow", bufs=2)
        nc.vector.tensor_mul(grow[:], fq[:], fk12[:])
        gb = wp.tile([96, H], F32, name="gb", tag="gb", bufs=2)
        nc.gpsimd.partition_broadcast(gb[:], grow[:], channels=96)
        st.kvp = [wp.tile([96, 96], BF16, name=f"kvp_{h}", tag=f"kvp_{h}")
                  for h in range(H)]
        for h in range(H):
            nc.vector.tensor_scalar_mul(st.kvp[h][:], st.kvu[h % KVH][:],
                                        gb[:, h:h + 1])

    def ph_e2(c2):
        if c2 == 0:
            st.OT = [wp.tile([128, N], BF16, name=f"OT_{j}", tag=f"OT_{j}")
                     for j in range(NK)]
        for h in range(H):
            pieces = _head_pieces(h)
            if len(pieces) == 1:
                j0, r00, _, _ = pieces[0]
                rhs = st.q3[j0][r00:r00 + 96, 512 * c2:512 * (c2 + 1)]
            else:
                qh = wp.tile([96, 512], BF16, name="qh", tag="qh", bufs=4)
                for pi, (j, r0, rr, cnt) in enumerate(pieces):
                    src_ap = st.q3[j][r0:r0 + cnt, 512 * c2:512 * (c2 + 1)]
                    if (h + pi) % 2 == 0:
                        nc.vector.tensor_copy(qh[rr:rr + cnt, :], src_ap)
                    else:
                        nc.scalar.copy(qh[rr:rr + cnt, :], src_ap)
                rhs = qh[:]
            pa_t = pa.tile([96, 512], F32, name="pat", tag="pa", bufs=2)
            nc.tensor.matmul(pa_t[:], st.kvp[h][:], rhs, start=True,
                             stop=True)
            if len(pieces) == 1:
                j0, r00, _, _ = pieces[0]
                nc.vector.tensor_tensor(
                    out=st.OT[j0][r00:r00 + 96, 512 * c2:512 * (c2 + 1)],
                    in0=pa_t[:],
                    in1=st.vdwc[h % KVH][:, 512 * c2:512 * (c2 + 1)],
                    op=ALU.add)
            else:
                pac = wp.tile([96, 512], BF16, name="pac", tag="pac", bufs=4)
                nc.scalar.copy(pac[:], pa_t[:])
                for (j, r0, rr, cnt) in pieces:
                    nc.vector.tensor_tensor(
                        out=st.OT[j][r0:r0 + cnt, 512 * c2:512 * (c2 + 1)],
                        in0=pac[rr:rr + cnt, :],
                        in1=st.vdwc[h % KVH][rr:rr + cnt,
                                             512 * c2:512 * (c2 + 1)],
                        op=ALU.add)

    def ph_proj(ta, tb):
        for t in range(ta, tb):
            for oc in range(3):
                py = pmm.tile([128, 384], F32, name="py", tag="pdw", bufs=2)
                for j in range(NK):
                    nc.tensor.matmul(py[:], st.OT[j][:, 128 * t:128 * (t + 1)],
                                     cn.PWT[j][:, 384 * oc:384 * (oc + 1)],
                                     start=(j == 0), stop=(j == NK - 1))
                ysb = wp.tile([128, 384], F32, name="ysb", tag="ysb", bufs=3)
                # bias rides the psum evacuation (pjb_bc pre-broadcast on host)
                nc.vector.tensor_tensor(out=ysb[:], in0=py[:],
                                        in1=cn.pjb_bc[:, 384 * oc:384 * (oc + 1)],
                                        op=ALU.add)
                t0 = b * N + 128 * t
                nc.sync.dma_start(out=cn.y_out[t0:t0 + 128,
                                               384 * oc:384 * (oc + 1)],
                                  in_=ysb[:])

    return dict(k_begin=ph_k_begin, k_tile=ph_k_tile, v_tile=ph_v_tile,
                v_end=ph_v_end, vt=ph_vt, e1=ph_e1, q=ph_q,
                dwc=ph_dwc, norms=ph_norms, e2=ph_e2, proj=ph_proj)


_NC_CACHE = None


def _get_nc():
    global _NC_CACHE
    if _NC_CACHE is None:
        _NC_CACHE = _build_kernel()
    return _NC_CACHE


def _hi_lo(a):
    hi = a.astype(_F8)
    lo = (a - hi.astype(np.float32)).astype(_F8)
    return hi, lo


def _host_consts(wq_w, wq_b, wkv_w, wkv_b, dwc_w, dwc_b, proj_w, proj_b):
    wqT = np.ascontiguousarray(np.asarray(wq_w, np.float32).T) * SW      # [in, out]
    wkvT = np.ascontiguousarray(np.asarray(wkv_w, np.float32).T) * SW    # [in, 768]
    qhi, qlo = _hi_lo(wqT)
    khi, klo = _hi_lo(wkvT)

    # wqfj: [128, j, slot(hi,lo), k, 128]
    wqfj = np.zeros((128, NK, 2, NK, 128), _F8)
    for k in range(NK):
        for j in range(NK):
            wqfj[:, j, 0, k, :] = qhi[128 * k:128 * (k + 1), 128 * j:128 * (j + 1)]
            wqfj[:, j, 1, k, :] = qlo[128 * k:128 * (k + 1), 128 * j:128 * (j + 1)]
    wq8r = np.zeros((128, 2, DIM), _F8)
    wq8r[:, 0, :] = qlo[128 * 8:, :]
    wq8r[:, 1, :] = qhi[128 * 8:, :]

    wkvf = np.zeros((128, 2, NK, 768), _F8)
    for k in range(NK):
        wkvf[:, 0, k, :] = khi[128 * k:128 * (k + 1), :]
        wkvf[:, 1, k, :] = klo[128 * k:128 * (k + 1), :]
    wkv8r = np.zeros((128, 2, 768), _F8)
    wkv8r[:, 0, :] = klo[128 * 8:, :]
    wkv8r[:, 1, :] = khi[128 * 8:, :]
    wkvfk = np.ascontiguousarray(wkvf[:, :, :, 0:384])
    wkvfv = np.ascontiguousarray(wkvf[:, :, :, 384:768])
    wkv8rk = np.ascontiguousarray(wkv8r[:, :, 0:384])
    wkv8rv = np.ascontiguousarray(wkv8r[:, :, 384:768])

    pwT = np.ascontiguousarray(np.asarray(proj_w, np.float32).T).astype(_BF)
    wqb = np.ascontiguousarray(np.asarray(wq_b, np.float32).reshape(NK, 128).T)
    kvb64 = (np.asarray(wkv_b, np.float32).reshape(1, 768) * SW).astype(_BF)
    pjb_bc = np.broadcast_to(np.asarray(proj_b, np.float32).reshape(1, DIM),
                             (128, DIM)).astype(_BF)

    dw = np.asarray(dwc_w, np.float32).reshape(KVH, 96, 9) * SW  # [g, d, tap]
    dgp = np.zeros((96, 2, KVH, 3, 96), np.float32)
    dge = np.zeros((96, KVH, 9, 96), np.float32)
    for d in range(96):
        for dxi in range(3):
            dgp[d, 0, :, dxi, d] = dw[:, d, 0 + dxi]       # dy=-1 taps 0,1,2
            dgp[d, 1, :, dxi, d] = dw[:, d, 6 + dxi]       # dy=+1 taps 6,7,8
        for ti in range(9):
            dge[d, :, ti, d] = dw[:, d, ti]
    dgp = dgp.astype(_F8)
    dge = dge.astype(_F8)
    dwcb = np.ascontiguousarray(np.asarray(dwc_b, np.float32).reshape(KVH, 96).T)

    mk = np.zeros((128, NK, H), np.float32)
    for j in range(NK):
        for p in range(128):
            f = 128 * j + p
            mk[p, j, f // 96] = 1.0
    masks = mk.astype(_BF)
    eye = np.eye(128, dtype=np.float32).astype(_BF)
    return dict(wqfj=wqfj, wq8r=wq8r, wkvfk=wkvfk, wkvfv=wkvfv, wkv8rk=wkv8rk,
                wkv8rv=wkv8rv, pwT=pwT, wqb=wqb, kvb64=kvb64, pjb_bc=pjb_bc,
                dgp=dgp, dge=dge, dwcb=dwcb, masks=masks, eye=eye)


def kernel(x, wq_w, wq_b, wkv_w, wkv_b, dwc_w, dwc_b, proj_w, proj_b,
           _want_results=False, **_unused):
    nc = _get_nc()
    consts = _host_consts(wq_w, wq_b, wkv_w, wkv_b, dwc_w, dwc_b, proj_w, proj_b)
    x = np.asarray(x, np.float32)
    in_maps = []
    for c in range(NCORES):
        m = dict(consts)
        m["x"] = np.ascontiguousarray(x[BL * c:BL * (c + 1)].reshape(T, DIM))
        in_maps.append(m)
    res = bass_utils.run_bass_kernel_spmd(nc, in_maps, core_ids=list(range(NCORES)))
    y = np.stack([res.results[c]["y"].reshape(BL, N, DIM) for c in range(NCORES)])
    y = y.reshape(B, N, DIM)
    if _want_results:
        return y, res
    return y
